# revision 1
# baseline (speedup 1.0000x reference)
"""Trainium2 Bass kernel for PointConv-style e3nn message passing.

Self-contained: builds + runs an 8-core SPMD Bass kernel via
bass_utils.run_bass_kernel_spmd, accepting FULL inputs and returning the
FULL output.

Design (v2):
- Nodes padded to 20480, split 8 ways (2560/core); edges sorted by dst and
  assigned to the core owning the destination.
- Per core, destinations are processed in 5 groups of 512 nodes. Edges of a
  group are packed into T 128-edge tiles on a uniform column grid (each tile
  owns a baked 32-column window of the group's 512 psum columns), so the
  scatter-add becomes per-tile compact one-hot matmuls into 5 psum banks.
- The a0/a1 spherical-harmonic factors are folded into host-prescaled
  one-hots (oh*a0, oh*a1_d), so the device only forms the w*g products.
- h = lin1(x) is computed replicated per-core into DRAM (node-major bf16)
  and per-edge rows are fetched with one big dma_gather per group.
"""

import os
import sys
import types
import ctypes

import numpy as np

import concourse.bass as bass
import concourse.bacc as bacc
import concourse.tile as tile
from concourse import mybir
from concourse.bass import AP
from concourse.bass_utils import run_bass_kernel_spmd
from concourse.library_config import mlp as _mlp_lib

# ---------------------------------------------------------------- constants
N = 20000
E = 160000
MUL = 64
EDIM = 8
NZ = 4
AVG_NEIGH = 8.0
INV_SQRT3 = float(1.0 / np.sqrt(3.0))

CORES = 8
NP_PAD = 20480            # padded node count
NPC = NP_PAD // CORES     # 2560 nodes per core
GRP = 512                 # nodes per scatter group (psum bank width)
NG = NPC // GRP           # 5 groups per core
SPAN = 32                 # onehot column window per edge tile
BACK = 8                  # grid look-back
STW = 8                   # supertile width (tiles per DVE batch)

F32 = mybir.dt.float32
BF16 = mybir.dt.bfloat16
I32 = mybir.dt.int32
I16 = mybir.dt.int16
NP_BF16 = mybir.dt.np(mybir.dt.bfloat16)

LAST_RESULT = None


# ------------------------------------------------------- axon profile hook
def _install_profile_hook():
    """Make trace=True / BASS_TRACE=1 work under axon (degrades silently)."""
    if "antenv.axon_hooks" in sys.modules:
        return
    try:
        try:
            from trn_agent_boot.trn_boot import _ntff_profile_via_ctypes
        except ImportError:
            sys.path.insert(0, "/root/.axon_site")
            from trn_agent_boot.trn_boot import _ntff_profile_via_ctypes
        so_path = "/opt/axon/libaxon_pjrt.so"
        lib = ctypes.CDLL(so_path)
        if not hasattr(lib, "axon_start_nrt_profile"):
            return
        hook = _ntff_profile_via_ctypes(so_path)
        mod = types.ModuleType("antenv.axon_hooks")
        state = {"hook": hook}
        mod.set_axon_ntff_profile_hook = lambda h: state.__setitem__("hook", h)
        mod.get_axon_ntff_profile_hook = lambda: state["hook"]
        sys.modules["antenv.axon_hooks"] = mod
        import antenv
        antenv.axon_hooks = mod
    except Exception:
        pass


# ----------------------------------------------- tile-exit drain workaround
def _patch_tile_drain():
    """This toolchain's walrus rejects >1 sem wait on a Drain; hang the exit
    waits on a NoOp chain instead (bacc's generate_event_semaphores then
    legalises them)."""
    from concourse.vector_clock import ScopedClock

    def _drain_and_barrier(self, tick_clock, wait_clock):
        nop_inst = self.nc.sync.nop(nofuse=True, hint="tile_exit_wait")
        wait_clock.add_sem_waits(
            nop_inst.ins, ScopedClock({None: tick_clock.global_clock})
        )
        self.nc.sync.drain()
        self.nc.all_engine_barrier()
        assert self.sems is not None
        popped = self.nc._tile_sem_poison_stack.pop()
        assert popped is self._sem_poison
        self.nc.clear_and_free_semaphores(list(self.sems.allocated().values()))
        self.nc.all_engine_barrier()

    tile.TileContext._drain_and_barrier = _drain_and_barrier


_patch_tile_drain()


def _grid_starts(T):
    return [max(0, min(int(round(i * GRP / T)) - BACK, GRP - SPAN))
            for i in range(T)]


def _apv(base_ap, col_off, dims):
    """AP view of a 2D sbuf/psum tile: partitions x custom free dims."""
    pstep, pcount = base_ap.ap[0]
    return AP(base_ap.tensor, base_ap.offset + col_off,
              [[pstep, pcount]] + dims)


# ---------------------------------------------------------------- program
def _build_program(T):
    """Build the SPMD Bass program for T edge tiles per 512-node group."""
    C = T * 128               # edge slots per group
    IDXW = C // 16            # idx table cols per group
    skip = set(os.environ.get("BASS_SKIP", "").split(","))

    nc = bacc.Bacc(num_swdge_queues=4)

    # inputs (per core)
    xTf = nc.dram_tensor("xTf", [256, NP_PAD], BF16, kind="ExternalInput")
    xTb = nc.dram_tensor("xTb", [256, NPC], BF16, kind="ExternalInput")
    arepb = nc.dram_tensor("arepb", [256, NPC], BF16, kind="ExternalInput")
    sidx = nc.dram_tensor("sidx", [128, NG * IDXW], I16, kind="ExternalInput")
    ohd = nc.dram_tensor("ohd", [NG, 128, T * 4 * SPAN], BF16,
                         kind="ExternalInput")
    embd = nc.dram_tensor("embd", [NG, EDIM, C], BF16, kind="ExternalInput")
    wblk1_d = nc.dram_tensor("wblk1", [128, 128], BF16, kind="ExternalInput")
    wblk2_d = nc.dram_tensor("wblk2", [128, 128], BF16, kind="ExternalInput")
    wm1_d = nc.dram_tensor("wm1", [EDIM, EDIM], BF16, kind="ExternalInput")
    wbig_d = nc.dram_tensor("wbig", [EDIM, 256], BF16, kind="ExternalInput")
    w20a_d = nc.dram_tensor("w20a_p", [128, 128], BF16, kind="ExternalInput")
    w20b_d = nc.dram_tensor("w20b_p", [128, 128], BF16, kind="ExternalInput")
    w21at_d = nc.dram_tensor("w21a_t", [128, 64], BF16, kind="ExternalInput")
    w21bt_d = nc.dram_tensor("w21b_t", [128, 64], BF16, kind="ExternalInput")
    w21bb_d = nc.dram_tensor("w21b_b", [128, 64], BF16, kind="ExternalInput")
    wsc0_d = nc.dram_tensor("wsc0", [2, 128, 128], BF16, kind="ExternalInput")
    wsc1_d = nc.dram_tensor("wsc1", [2, 128, 64], BF16, kind="ExternalInput")
    outT = nc.dram_tensor("outT", [256, NPC], BF16, kind="ExternalOutput")

    ACT_SILU = mybir.ActivationFunctionType.Silu
    ACT_COPY = mybir.ActivationFunctionType.Copy
    MULT = mybir.AluOpType.mult

    starts = _grid_starts(T)

    with tile.TileContext(nc) as tc:
        with (
            tc.tile_pool(name="const", bufs=1) as cp,
            tc.tile_pool(name="hx", bufs=2) as hxp,
            tc.tile_pool(name="hout", bufs=6) as hop,
            tc.tile_pool(name="grp", bufs=2) as gp,
            tc.tile_pool(name="hid5", bufs=5) as hp5,
            tc.tile_pool(name="st", bufs=3) as sp,
            tc.tile_pool(name="sts", bufs=2) as stsp,
            tc.tile_pool(name="node", bufs=2) as npl,
            tc.tile_pool(name="bank", bufs=1, space="PSUM") as bkp,
            tc.tile_pool(name="wps", bufs=2, space="PSUM") as wpp,
            tc.tile_pool(name="mps", bufs=1, space="PSUM") as mpp,
            tc.tile_pool(name="dram", bufs=1, space="DRAM") as dp,
        ):
            if "lib" not in skip:
                nc.gpsimd.load_library(_mlp_lib)

            hdram = dp.tile([NP_PAD, 256], BF16)

            # ---- constants
            wblk1 = cp.tile([128, 128], BF16)
            nc.sync.dma_start(out=wblk1[:], in_=wblk1_d[:])
            wblk2 = cp.tile([128, 128], BF16)
            nc.sync.dma_start(out=wblk2[:], in_=wblk2_d[:])
            wm1 = cp.tile([EDIM, EDIM], BF16)
            nc.sync.dma_start(out=wm1[:], in_=wm1_d[:])
            wbig = cp.tile([EDIM, 256], BF16)
            nc.sync.dma_start(out=wbig[:], in_=wbig_d[:])
            w20a_p = cp.tile([128, 128], BF16)
            nc.sync.dma_start(out=w20a_p[:], in_=w20a_d[:])
            w20b_p = cp.tile([128, 128], BF16)
            nc.sync.dma_start(out=w20b_p[:], in_=w20b_d[:])
            w21a_t = cp.tile([128, 64], BF16)
            nc.sync.dma_start(out=w21a_t[:], in_=w21at_d[:])
            w21b_t = cp.tile([128, 64], BF16)
            nc.sync.dma_start(out=w21b_t[:], in_=w21bt_d[:])
            w21b_b = cp.tile([128, 64], BF16)
            nc.sync.dma_start(out=w21b_b[:], in_=w21bb_d[:])
            wsc0a = cp.tile([128, 128], BF16)
            nc.sync.dma_start(out=wsc0a[:], in_=wsc0_d[0])
            wsc0b = cp.tile([128, 128], BF16)
            nc.sync.dma_start(out=wsc0b[:], in_=wsc0_d[1])
            wsc1a = cp.tile([128, 64], BF16)
            nc.sync.dma_start(out=wsc1a[:], in_=wsc1_d[0])
            wsc1b = cp.tile([128, 64], BF16)
            nc.sync.dma_start(out=wsc1b[:], in_=wsc1_d[1])
            idxt = cp.tile([128, NG * IDXW], I16)
            nc.sync.dma_start(out=idxt[:], in_=sidx[:])
            zeros = cp.tile([128, 512], BF16)
            nc.vector.memset(zeros[:], 0.0)

            # ============ radial MLP for all groups (independent of h) ====
            hids = []
            for g in range(NG):
                embt = gp.tile([EDIM, C], BF16, tag="embt")
                nc.sync.dma_start(out=embt[:], in_=embd[g])
                hid = hp5.tile([EDIM, C], BF16, tag="hid", name=f"hid{g}")
                for c0 in range(0, C, 512):
                    sz = min(512, C - c0)
                    hp = wpp.tile([EDIM, 512], F32, tag="wpb")
                    nc.tensor.matmul(hp[:, :sz], lhsT=wm1[:],
                                     rhs=embt[:, c0:c0 + sz],
                                     start=True, stop=True)
                    nc.scalar.activation(hid[:, c0:c0 + sz], hp[:, :sz],
                                         ACT_SILU)
                hids.append(hid)

            # ================= phase H: h = lin1(x) table =================
            # hdram physical layout: [40 blocks of 512 nodes][128 p][4 q][256]
            # node n -> row r = (n>>9)*512 + (n&127)*4 + ((n>>7)&3)
            hcnt = 0
            for jj in range(0, NP_PAD, 4096):
                xa = hxp.tile([128, 4096], BF16, tag="xa")
                nc.gpsimd.dma_start(out=xa[:], in_=xTf[0:128, jj:jj + 4096])
                xb = hxp.tile([128, 4096], BF16, tag="xb")
                nc.gpsimd.dma_start(out=xb[:], in_=xTf[128:256, jj:jj + 4096])
                for q in range(8):
                    hsb = hop.tile([128, 1024], BF16, tag="hsb")
                    for half in range(2):
                        c0 = 512 * q + 256 * half
                        hppb = bkp.tile([128, 512], F32,
                                        tag=f"bank{hcnt % 5}",
                                        name=f"hppb{hcnt}")
                        nc.tensor.matmul(hppb[:, 0:128],
                                         lhsT=xa[:, c0:c0 + 128],
                                         rhs=wblk1[:], start=True, stop=True)
                        nc.tensor.matmul(hppb[:, 128:256],
                                         lhsT=xb[:, c0:c0 + 128],
                                         rhs=wblk2[:], start=True, stop=True)
                        nc.tensor.matmul(hppb[:, 256:384],
                                         lhsT=xa[:, c0 + 128:c0 + 256],
                                         rhs=wblk1[:], start=True, stop=True)
                        nc.tensor.matmul(hppb[:, 384:512],
                                         lhsT=xb[:, c0 + 128:c0 + 256],
                                         rhs=wblk2[:], start=True, stop=True)
                        if hcnt % 2 == 0:
                            nc.scalar.activation(hsb[:, 512 * half:
                                                     512 * half + 512],
                                                 hppb[:], ACT_COPY)
                        else:
                            nc.vector.tensor_copy(
                                out=hsb[:, 512 * half:512 * half + 512],
                                in_=hppb[:])
                        hcnt += 1
                    block = (jj + 512 * q) // 512
                    hfull = hdram[:]
                    hd3 = AP(hfull.tensor, hfull.offset + block * 512 * 256,
                             [[1024, 128], [256, 4], [1, 256]])
                    nc.sync.dma_start(out=hd3, in_=hsb[:])

            # scatter psum banks (reused across groups)
            def bank_tiles():
                return [bkp.tile([128, 512], F32, tag=f"bank{k}",
                                 name=f"bank{k}") for k in range(5)]

            # ================= per-group edge + node phases =================
            st_list = [(g, st) for g in range(NG) for st in range(0, T, STW)]
            hsg_tiles = {}
            gq_state = [0]

            def emit_gather(i):
                if i >= len(st_list):
                    return
                gg, sst = st_list[i]
                LL = min(STW, T - sst)
                hsg = sp.tile([128, STW * 256], BF16, tag="hsg", bufs=4,
                              name=f"hsg_{gg}_{sst}")
                hs_ap = AP(hsg.tensor, hsg.offset,
                           [hsg.ap[0], [256, LL], [1, 256]])
                nc.gpsimd.dma_gather(
                    out_ap=hs_ap, in_ap=hdram[:],
                    idxs_ap=idxt[:, gg * IDXW + sst * 8:
                                 gg * IDXW + (sst + LL) * 8],
                    num_idxs=LL * 128, num_idxs_reg=LL * 128,
                    elem_size=256, queue_num=gq_state[0] % 4)
                gq_state[0] += 1
                hsg_tiles[i] = hsg

            GLOOK = 1
            for i in range(GLOOK):
                emit_gather(i)
            sti = 0
            for g in range(NG):
                banks = bank_tiles()
                if "memset" not in skip:
                    for k in range(5):
                        if k % 2 == 0:
                            nc.vector.memset(banks[k][:], 0.0)
                        else:
                            nc.scalar.activation(banks[k][:], zeros[:],
                                                 ACT_COPY)

                hid = hids[g]

                # prefetch node-phase inputs for this group
                cols = slice(g * GRP, (g + 1) * GRP)
                xga = npl.tile([128, 512], BF16, tag="xga")
                nc.sync.dma_start(out=xga[:], in_=xTb[0:128, cols])
                xgb = npl.tile([128, 512], BF16, tag="xgb")
                nc.sync.dma_start(out=xgb[:], in_=xTb[128:256, cols])
                ara = npl.tile([128, 512], BF16, tag="ara")
                nc.sync.dma_start(out=ara[:], in_=arepb[0:128, cols])
                arb = npl.tile([128, 512], BF16, tag="arb")
                nc.sync.dma_start(out=arb[:], in_=arepb[128:256, cols])
                x4s = []
                for d in range(4):
                    x4t = npl.tile([128, 512], BF16, tag=f"x4_{d}",
                                   name=f"x4_{d}_{g}")
                    xsl = xTb[64 * d:64 * d + 64, cols]
                    nc.sync.dma_start(out=x4t[:], in_=AP(
                        xsl.tensor, xsl.offset, [[0, 2]] + xsl.ap))
                    x4s.append(x4t)

                # edge supertiles
                for st in range(0, T, STW):
                    L = min(STW, T - st)
                    emit_gather(sti + GLOOK)
                    hsg = hsg_tiles.pop(sti)
                    sti += 1
                    oht = sp.tile([128, STW * 4 * SPAN], BF16, tag="oht")
                    nc.sync.dma_start(
                        out=oht[:, :L * 4 * SPAN],
                        in_=ohd[g][:, st * 4 * SPAN:(st + L) * 4 * SPAN])

                    # per-edge tp weights wp = hid_t^T @ wbig  [128e, 256]
                    wp_sb = sp.tile([128, STW * 256], BF16, tag="wp_sb")
                    for pq in ([] if "wp" in skip else range(0, L, 2)):
                        wpb = wpp.tile([128, 512], F32, tag="wpb")
                        for i in range(min(2, L - pq)):
                            t = st + pq + i
                            nc.tensor.matmul(
                                wpb[:, 256 * i:256 * i + 256],
                                lhsT=hid[:, 128 * t:128 * t + 128],
                                rhs=wbig[:], start=True, stop=True)
                        nc.scalar.activation(
                            wp_sb[:, 256 * pq:256 * (pq + min(2, L - pq))],
                            wpb[:, :256 * min(2, L - pq)], ACT_COPY)

                    # wg products: [A|B0|B1|B2|C|D0|C|D1|C|D2] per tile
                    # layout: 640 cols/tile: A(64) B_d(192) [C|D_d](3x128)
                    wgt = sp.tile([128, STW * 640], BF16, tag="wgt", bufs=2)
                    hoff = 0
                    if "wg" in skip:
                        pass
                    else:
                      # A = w1*g0
                      nc.vector.tensor_tensor(
                        out=_apv(wgt[:], 0, [[640, L], [1, 64]]),
                        in0=_apv(wp_sb[:], 0, [[256, L], [1, 64]]),
                        in1=_apv(hsg[:], hoff, [[256, L], [1, 64]]),
                        op=MULT)
                      # B_d = w3*g1_d
                      nc.vector.tensor_tensor(
                        out=_apv(wgt[:], 64, [[640, L], [64, 3], [1, 64]]),
                        in0=_apv(wp_sb[:], 64, [[256, L], [0, 3], [1, 64]]),
                        in1=_apv(hsg[:], hoff + 64,
                                 [[256, L], [64, 3], [1, 64]]),
                        op=MULT)
                      # C = w2*g0 (replicated 3x at 256+128d)
                      nc.vector.tensor_tensor(
                        out=_apv(wgt[:], 256, [[640, L], [128, 3], [1, 64]]),
                        in0=_apv(wp_sb[:], 128, [[256, L], [0, 3], [1, 64]]),
                        in1=_apv(hsg[:], hoff, [[256, L], [0, 3], [1, 64]]),
                        op=MULT)
                      # D_d = w4'*g1_d (at 320+128d)
                      nc.vector.tensor_tensor(
                        out=_apv(wgt[:], 320, [[640, L], [128, 3], [1, 64]]),
                        in0=_apv(wp_sb[:], 192, [[256, L], [0, 3], [1, 64]]),
                        in1=_apv(hsg[:], hoff + 64,
                                 [[256, L], [64, 3], [1, 64]]),
                        op=MULT)

                    # compact scatter matmuls
                    for lt in ([] if "scatter" in skip else range(L)):
                        t = st + lt
                        col0 = starts[t]
                        wb = lt * 640
                        ob = lt * 4 * SPAN
                        last = (t == T - 1)
                        # bankP: [s0a|s1b_0] <- [A|B0] x oh_a0
                        nc.tensor.matmul(
                            banks[0][:, col0:col0 + SPAN],
                            lhsT=wgt[:, wb:wb + 128],
                            rhs=oht[:, ob:ob + SPAN],
                            start=False, stop=last, skip_group_check=True)
                        # bankQ: [s1b_1|s1b_2] <- [B1|B2] x oh_a0
                        nc.tensor.matmul(
                            banks[1][:, col0:col0 + SPAN],
                            lhsT=wgt[:, wb + 128:wb + 256],
                            rhs=oht[:, ob:ob + SPAN],
                            start=False, stop=last, skip_group_check=True)
                        # bankR_d: [s1a_d|s0b_d] <- [C|D_d] x oh_a1_d
                        for d in range(3):
                            nc.tensor.matmul(
                                banks[2 + d][:, col0:col0 + SPAN],
                                lhsT=wgt[:, wb + 256 + 128 * d:
                                         wb + 384 + 128 * d],
                                rhs=oht[:, ob + SPAN * (1 + d):
                                        ob + SPAN * (2 + d)],
                                start=False, stop=last,
                                skip_group_check=True)

                # drain banks -> sts (bf16)
                sts = []
                for k in range(5):
                    stile = stsp.tile([128, 512], BF16, tag=f"sts{k}",
                                      name=f"sts{k}_{g}")
                    if k % 2 == 0:
                        nc.scalar.activation(stile[:], banks[k][:], ACT_COPY)
                    else:
                        nc.vector.tensor_copy(out=stile[:], in_=banks[k][:])
                    sts.append(stile)
                stsP, stsQ, stsR = sts[0], sts[1], sts[2:5]

                # ---------------- node phase for this group ----------------
                pass

                if "node" in skip:
                    outa = npl.tile([128, 512], BF16, tag="outa")
                    nc.vector.tensor_copy(out=outa[:], in_=xga[:])
                    outb = npl.tile([128, 512], BF16, tag="outb")
                    nc.vector.tensor_copy(out=outb[:], in_=xgb[:])
                else:
                    # up0 = W20^T s0 + sc0   [scalars|gates, 512]
                    up0 = bkp.tile([128, 512], F32, tag="bank0",
                                   name=f"up0_{g}")
                    nc.tensor.matmul(up0[:], lhsT=w20a_p[:],
                                     rhs=stsP[:], start=True, stop=False)
                    for d in range(3):
                        nc.tensor.matmul(up0[:], lhsT=w20b_p[:],
                                         rhs=stsR[d][:],
                                         start=False, stop=False)
                    x4 = x4s[0]
                    ya = npl.tile([128, 512], BF16, tag="ya")
                    nc.vector.tensor_tensor(out=ya[:], in0=x4[:], in1=ara[:],
                                            op=MULT)
                    yb = npl.tile([128, 512], BF16, tag="yb")
                    nc.vector.tensor_tensor(out=yb[:], in0=x4[:], in1=arb[:],
                                            op=MULT)
                    nc.tensor.matmul(up0[:], lhsT=wsc0a[:], rhs=ya[:],
                                     start=False, stop=False)
                    nc.tensor.matmul(up0[:], lhsT=wsc0b[:], rhs=yb[:],
                                     start=False, stop=True)

                    # up1: d0 rows 0:64, d1 rows 64:128 of up1a; d2 in up1b
                    up1a = bkp.tile([128, 512], F32, tag="bank1",
                                     name=f"up1a_{g}")
                    for d in (0, 1):
                        rows = slice(64 * d, 64 * d + 64)
                        if d == 0:
                            s1b_src, s1b_w = stsP[:], w21b_b[:]
                        else:
                            s1b_src, s1b_w = stsQ[:], w21b_t[:]
                        nc.tensor.matmul(up1a[rows, :], lhsT=w21a_t[:],
                                         rhs=stsR[d][:],
                                         start=True, stop=False)
                        nc.tensor.matmul(up1a[rows, :], lhsT=s1b_w, rhs=s1b_src,
                                         start=False, stop=False)
                        x4d = x4s[1 + d]
                        yda = npl.tile([128, 512], BF16, tag="yda")
                        nc.vector.tensor_tensor(out=yda[:], in0=x4d[:],
                                                in1=ara[:], op=MULT)
                        ydb = npl.tile([128, 512], BF16, tag="ydb")
                        nc.vector.tensor_tensor(out=ydb[:], in0=x4d[:],
                                                in1=arb[:], op=MULT)
                        nc.tensor.matmul(up1a[rows, :], lhsT=wsc1a[:], rhs=yda[:],
                                         start=False, stop=False)
                        nc.tensor.matmul(up1a[rows, :], lhsT=wsc1b[:], rhs=ydb[:],
                                         start=False, stop=True)

                    # gate scalars/gates -> bf16
                    t0s = npl.tile([128, 512], BF16, tag="t0s")
                    nc.scalar.activation(t0s[:], up0[:], ACT_SILU)

                    up1b = mpp.tile([64, 512], F32, tag="mpsum")
                    nc.tensor.matmul(up1b[:], lhsT=w21a_t[:],
                                     rhs=stsR[2][:], start=True, stop=False)
                    nc.tensor.matmul(up1b[:], lhsT=w21b_b[:],
                                     rhs=stsQ[:], start=False, stop=False)
                    x4d2 = x4s[3]
                    yda2 = npl.tile([128, 512], BF16, tag="yda")
                    nc.vector.tensor_tensor(out=yda2[:], in0=x4d2[:], in1=ara[:],
                                            op=MULT)
                    ydb2 = npl.tile([128, 512], BF16, tag="ydb")
                    nc.vector.tensor_tensor(out=ydb2[:], in0=x4d2[:], in1=arb[:],
                                            op=MULT)
                    nc.tensor.matmul(up1b[:], lhsT=wsc1a[:], rhs=yda2[:],
                                     start=False, stop=False)
                    nc.tensor.matmul(up1b[:], lhsT=wsc1b[:], rhs=ydb2[:],
                                     start=False, stop=True)

                    # assemble: vectors = gates*t1 (t1 from psum); resnet add
                    outa = npl.tile([128, 512], BF16, tag="outa")
                    nc.vector.tensor_add(out=outa[0:64, :], in0=t0s[0:64, :],
                                         in1=xga[0:64, :])
                    nc.vector.tensor_tensor(out=outa[64:128, :],
                                            in0=t0s[64:128, :],
                                            in1=up1a[0:64, :], op=MULT)
                    nc.vector.tensor_add(out=outa[64:128, :], in0=outa[64:128, :],
                                         in1=xga[64:128, :])
                    outb = npl.tile([128, 512], BF16, tag="outb")
                    nc.vector.tensor_tensor(out=outb[0:64, :],
                                            in0=t0s[64:128, :],
                                            in1=up1a[64:128, :], op=MULT)
                    nc.vector.tensor_add(out=outb[0:64, :], in0=outb[0:64, :],
                                         in1=xgb[0:64, :])
                    nc.vector.tensor_tensor(out=outb[64:128, :],
                                            in0=t0s[64:128, :],
                                            in1=up1b[:], op=MULT)
                    nc.vector.tensor_add(out=outb[64:128, :], in0=outb[64:128, :],
                                         in1=xgb[64:128, :])

                nc.sync.dma_start(out=outT[0:128, cols], in_=outa[:])
                nc.sync.dma_start(out=outT[128:256, cols], in_=outb[:])

    nc.compile()
    return nc


# ---------------------------------------------------------------- host prep
def _pack_group(cols, T):
    """Greedy pack of sorted dst-cols into T tiles on the uniform grid.
    Returns per-tile edge index lists (positions into cols) or None."""
    starts = _grid_starts(T)
    res = []
    j, nE = 0, len(cols)
    for t in range(T):
        lo, hi = starts[t], starts[t] + SPAN
        tl = []
        while j < nE and len(tl) < 128 and cols[j] < hi:
            if cols[j] < lo:
                return None
            tl.append(j)
            j += 1
        res.append(tl)
    if j < nE:
        return None
    return res


def _host_prep(node_feats, node_attrs, edge_attrs, edge_embedding,
               W_lin1_0, W_lin1_1, W_mlp1, W_mlp2,
               W_lin2_0, W_lin2_1, W_sc0, W_sc1, edge_index):
    inv = 1.0 / np.sqrt(MUL)
    inv_e = 1.0 / np.sqrt(EDIM)
    inv2 = 1.0 / np.sqrt(2 * MUL)
    inv_n = 1.0 / np.sqrt(AVG_NEIGH)
    inv_sc = 1.0 / np.sqrt(MUL * NZ)

    # channel permutation: ours = [x0(64) | x1 d-major(192)]
    gidx = np.empty(256, np.int64)
    gidx[:64] = np.arange(64)
    for d in range(3):
        for u in range(64):
            gidx[64 + 64 * d + u] = 64 + 3 * u + d

    xgf = np.zeros((NP_PAD, 256), np.float32)
    xgf[:N] = node_feats[:, gidx]
    xT = np.ascontiguousarray(xgf.T)
    xTf = xT.astype(NP_BF16)

    arep_full = np.zeros((256, NP_PAD), np.float32)
    arep_full[:, :N] = np.repeat(node_attrs.T.astype(np.float32), MUL, axis=0)
    arepb_full = arep_full.astype(NP_BF16)

    # ---- edge sorting and per-(core,group) packing
    src = edge_index[0].astype(np.int64)
    dst = edge_index[1].astype(np.int64)
    order = np.argsort(dst, kind="stable")
    src_s, dst_s = src[order], dst[order]
    ea_s = edge_attrs[order].astype(np.float32)
    emb_s = edge_embedding[order].astype(np.float32)

    bounds = np.searchsorted(dst_s, np.arange(0, NP_PAD + 1, GRP))
    all_cols = []
    T = 2
    for c in range(CORES):
        for g in range(NG):
            gi = c * NG + g
            s, e = bounds[gi], bounds[gi + 1]
            cols = (dst_s[s:e] - gi * GRP).astype(int)
            all_cols.append(cols)
            Tg = max(1, int(np.ceil(len(cols) / 128)))
            while Tg < 96 and _pack_group(cols, Tg) is None:
                Tg += 1
            T = max(T, Tg)
    T = T + (T % 2)  # even for wp pairing
    while any(_pack_group(cols, T) is None for cols in all_cols):
        T += 2

    starts = _grid_starts(T)
    C = T * 128
    IDXW = C // 16

    per_core = []
    for c in range(CORES):
        sidx_flat = np.zeros((NG, C), np.int16)
        oh = np.zeros((NG, T, 128, 4 * SPAN), np.float32)
        embw = np.zeros((NG, EDIM, C), np.float32)
        for g in range(NG):
            gi = c * NG + g
            s = bounds[gi]
            cols = all_cols[gi]
            pk = _pack_group(cols, T)
            assert pk is not None
            for t, tl in enumerate(pk):
                if not tl:
                    continue
                idx = np.asarray(tl, np.int64)
                p = np.arange(len(tl))
                slot = t * 128 + p
                sn = src_s[s + idx]
                rmap = ((sn >> 9) * 512 + (sn & 127) * 4
                        + ((sn >> 7) & 3))
                sidx_flat[g, slot] = rmap.astype(np.int16)
                embw[g, :, slot] = emb_s[s + idx]
                cc = cols[idx] - starts[t]
                oh[g, t, p, cc] = ea_s[s + idx, 0]               # oh*a0
                for d in range(3):
                    oh[g, t, p, SPAN * (1 + d) + cc] = ea_s[s + idx, 1 + d]
        # gather idx layout: idx i at [i % 16 (+16k), i // 16]
        sidx16 = sidx_flat.reshape(NG, IDXW, 16).transpose(0, 2, 1)
        sidx128 = np.tile(sidx16, (1, 8, 1)).transpose(1, 0, 2).reshape(
            128, NG * IDXW)
        # device layout [NG, 128, T*4*SPAN]
        oh_dev = oh.transpose(0, 2, 1, 3).reshape(NG, 128, T * 4 * SPAN)
        per_core.append(dict(
            sidx=np.ascontiguousarray(sidx128),
            ohd=np.ascontiguousarray(oh_dev).astype(NP_BF16),
            embd=embw.astype(NP_BF16),
        ))

    # ---- weights
    W10s = (W_lin1_0 * inv).astype(np.float32)
    W11s = (W_lin1_1 * inv).astype(np.float32)
    wblk1 = np.zeros((128, 128), np.float32)
    wblk1[:64, :64] = W10s
    wblk1[64:, 64:] = W11s
    wblk2 = np.zeros((128, 128), np.float32)
    wblk2[:64, :64] = W11s
    wblk2[64:, 64:] = W11s
    wm1 = (W_mlp1 * inv_e).astype(NP_BF16)
    w1 = W_mlp2[:, 0:64]
    w2 = W_mlp2[:, 64:128]
    w3 = W_mlp2[:, 128:192]
    w4 = W_mlp2[:, 192:256]
    # wp cols: [w1 | w3 | w2 | w4']
    wbig = (np.concatenate([w1, w3, w2, w4 * INV_SQRT3], axis=1)
            * inv_e).astype(NP_BF16)
    w20s = (W_lin2_0 * inv2 * inv_n).astype(np.float32)
    w21s = (W_lin2_1 * inv2 * inv_n).astype(np.float32)
    z64x128 = np.zeros((64, 128), np.float32)
    z64x64 = np.zeros((64, 64), np.float32)
    w20a_p = np.concatenate([w20s[0:64], z64x128]).astype(NP_BF16)
    w20b_p = np.concatenate([z64x128, w20s[64:128]]).astype(NP_BF16)
    w21a_t = np.concatenate([w21s[0:64], z64x64]).astype(NP_BF16)
    w21b_t = np.concatenate([w21s[64:128], z64x64]).astype(NP_BF16)
    w21b_b = np.concatenate([z64x64, w21s[64:128]]).astype(NP_BF16)
    wsc0z = (np.transpose(W_sc0, (1, 0, 2)).reshape(NZ * MUL, 2 * MUL)
             * inv_sc).astype(NP_BF16)
    wsc1z = (np.transpose(W_sc1, (1, 0, 2)).reshape(NZ * MUL, MUL)
             * inv_sc).astype(NP_BF16)
    wsc0 = np.stack([wsc0z[:128], wsc0z[128:]])
    wsc1 = np.stack([wsc1z[:128], wsc1z[128:]])

    shared = dict(xTf=xTf, wblk1=wblk1.astype(NP_BF16),
                  wblk2=wblk2.astype(NP_BF16), wm1=wm1, wbig=wbig,
                  w20a_p=w20a_p, w20b_p=w20b_p, w21a_t=w21a_t,
                  w21b_t=w21b_t, w21b_b=w21b_b, wsc0=wsc0, wsc1=wsc1)
    in_maps = []
    for c in range(CORES):
        m = dict(shared)
        m["xTb"] = np.ascontiguousarray(xTf[:, c * NPC:(c + 1) * NPC])
        m["arepb"] = np.ascontiguousarray(
            arepb_full[:, c * NPC:(c + 1) * NPC])
        m.update(per_core[c])
        in_maps.append(m)
    return T, in_maps, gidx


_PROGRAM_CACHE = {}


def kernel(**inputs):
    global LAST_RESULT
    _install_profile_hook()

    args = {k: np.asarray(v) for k, v in inputs.items()}
    T, in_maps, gidx = _host_prep(
        args["node_feats"].astype(np.float32),
        args["node_attrs"].astype(np.float32),
        args["edge_attrs"].astype(np.float32),
        args["edge_embedding"].astype(np.float32),
        args["W_lin1_0"].astype(np.float32),
        args["W_lin1_1"].astype(np.float32),
        args["W_mlp1"].astype(np.float32),
        args["W_mlp2"].astype(np.float32),
        args["W_lin2_0"].astype(np.float32),
        args["W_lin2_1"].astype(np.float32),
        args["W_sc0"].astype(np.float32),
        args["W_sc1"].astype(np.float32),
        args["edge_index"])

    if T not in _PROGRAM_CACHE:
        _PROGRAM_CACHE[T] = _build_program(T)
    nc = _PROGRAM_CACHE[T]

    trace = bool(int(os.environ.get("BASS_TRACE", "0")))
    res = run_bass_kernel_spmd(nc, in_maps, core_ids=list(range(CORES)),
                               trace=trace)
    LAST_RESULT = res

    outT = np.concatenate(
        [res.results[c]["outT"].astype(np.float32) for c in range(CORES)],
        axis=1)                            # [256, NP_PAD]
    full = outT.T[:N]                      # [N, 256] in our channel order
    out = np.empty((N, 256), np.float32)
    out[:, gidx] = full
    return out



# revision 8
# speedup vs baseline: 1.0291x; 1.0291x over previous
"""Trainium2 Bass kernel for PointConv-style e3nn message passing.

Self-contained: builds + runs an 8-core SPMD Bass kernel via
bass_utils.run_bass_kernel_spmd, accepting FULL inputs and returning the
FULL output.

Design (v3):
- Nodes padded to 20480, split 8 ways (2560/core); edges sorted by dst and
  assigned to the core owning the destination.
- Per core, destinations are processed in 5 groups of 512 nodes. Edges of a
  group are packed into T 128-edge tiles on a uniform column grid (each tile
  owns a baked 32-column window of the group's 512 psum columns), so the
  scatter-add becomes per-tile compact one-hot matmuls into 5 psum banks.
- The a0/a1 spherical-harmonic factors are folded into host-prescaled
  one-hots (oh*a0, oh*a1_d), so the device only forms the w*g products.
- v3 change vs v2: no replicated h-table and no device dma_gather. The host
  pre-gathers x[src] per edge slot (channel-major, slot order) and the
  device computes h = lin1(x) per 128-edge tile with two blockdiag matmuls
  directly into psum, alongside the radial tp-weight matmul.
"""

import os
import sys
import types
import ctypes

import numpy as np

import concourse.bass as bass
import concourse.bacc as bacc
import concourse.tile as tile
from concourse import mybir
from concourse.bass import AP
from concourse.bass_utils import run_bass_kernel_spmd

# ---------------------------------------------------------------- constants
N = 20000
E = 160000
MUL = 64
EDIM = 8
NZ = 4
AVG_NEIGH = 8.0
INV_SQRT3 = float(1.0 / np.sqrt(3.0))

CORES = 8
NP_PAD = 20480            # padded node count
NPC = NP_PAD // CORES     # 2560 nodes per core
GRP = 512                 # nodes per scatter group (psum bank width)
NG = NPC // GRP           # 5 groups per core
SPAN = 32                 # onehot column window per edge tile
BACK = 8                  # grid look-back
STW = 8                   # supertile width (tiles per DVE batch)

F32 = mybir.dt.float32
BF16 = mybir.dt.bfloat16
I32 = mybir.dt.int32
NP_BF16 = mybir.dt.np(mybir.dt.bfloat16)

LAST_RESULT = None


# ------------------------------------------------------- axon profile hook
def _install_profile_hook():
    """Make trace=True / BASS_TRACE=1 work under axon (degrades silently)."""
    if "antenv.axon_hooks" in sys.modules:
        return
    try:
        try:
            from trn_agent_boot.trn_boot import _ntff_profile_via_ctypes
        except ImportError:
            sys.path.insert(0, "/root/.axon_site")
            from trn_agent_boot.trn_boot import _ntff_profile_via_ctypes
        so_path = "/opt/axon/libaxon_pjrt.so"
        lib = ctypes.CDLL(so_path)
        if not hasattr(lib, "axon_start_nrt_profile"):
            return
        hook = _ntff_profile_via_ctypes(so_path)
        mod = types.ModuleType("antenv.axon_hooks")
        state = {"hook": hook}
        mod.set_axon_ntff_profile_hook = lambda h: state.__setitem__("hook", h)
        mod.get_axon_ntff_profile_hook = lambda: state["hook"]
        sys.modules["antenv.axon_hooks"] = mod
        import antenv
        antenv.axon_hooks = mod
    except Exception:
        pass


# ----------------------------------------------- tile-exit drain workaround
def _patch_tile_drain():
    """This toolchain's walrus rejects >1 sem wait on a Drain; hang the exit
    waits on a NoOp chain instead (bacc's generate_event_semaphores then
    legalises them)."""
    from concourse.vector_clock import ScopedClock

    def _drain_and_barrier(self, tick_clock, wait_clock):
        nop_inst = self.nc.sync.nop(nofuse=True, hint="tile_exit_wait")
        wait_clock.add_sem_waits(
            nop_inst.ins, ScopedClock({None: tick_clock.global_clock})
        )
        self.nc.sync.drain()
        self.nc.all_engine_barrier()
        assert self.sems is not None
        popped = self.nc._tile_sem_poison_stack.pop()
        assert popped is self._sem_poison
        self.nc.clear_and_free_semaphores(list(self.sems.allocated().values()))
        self.nc.all_engine_barrier()

    tile.TileContext._drain_and_barrier = _drain_and_barrier


_patch_tile_drain()


def _grid_starts(T):
    return [max(0, min(int(round(i * GRP / T)) - BACK, GRP - SPAN))
            for i in range(T)]


def _apv(base_ap, col_off, dims):
    """AP view of a 2D sbuf/psum tile: partitions x custom free dims."""
    pstep, pcount = base_ap.ap[0]
    return AP(base_ap.tensor, base_ap.offset + col_off,
              [[pstep, pcount]] + dims)


# ---------------------------------------------------------------- program
def _build_program(T):
    """Build the SPMD Bass program for T edge tiles per 512-node group."""
    C = T * 128               # edge slots per group
    skip = set(os.environ.get("BASS_SKIP", "").split(","))

    nc = bacc.Bacc(num_swdge_queues=4)

    # inputs (per core)
    xeT = nc.dram_tensor("xeT", [NG, 256, C], BF16, kind="ExternalInput")
    xTb = nc.dram_tensor("xTb", [256, NPC], BF16, kind="ExternalInput")
    arepb = nc.dram_tensor("arepb", [256, NPC], BF16, kind="ExternalInput")
    ohd = nc.dram_tensor("ohd", [NG, 128, T * 4 * SPAN], BF16,
                         kind="ExternalInput")
    embd = nc.dram_tensor("embd", [NG, EDIM, C], BF16, kind="ExternalInput")
    wblk1_d = nc.dram_tensor("wblk1", [128, 128], BF16, kind="ExternalInput")
    wblk2_d = nc.dram_tensor("wblk2", [128, 128], BF16, kind="ExternalInput")
    wm1_d = nc.dram_tensor("wm1", [EDIM, EDIM], BF16, kind="ExternalInput")
    wbig_d = nc.dram_tensor("wbig", [EDIM, 256], BF16, kind="ExternalInput")
    w20a_d = nc.dram_tensor("w20a_p", [128, 128], BF16, kind="ExternalInput")
    w20b_d = nc.dram_tensor("w20b_p", [128, 128], BF16, kind="ExternalInput")
    w21at_d = nc.dram_tensor("w21a_t", [128, 64], BF16, kind="ExternalInput")
    w21bt_d = nc.dram_tensor("w21b_t", [128, 64], BF16, kind="ExternalInput")
    w21bb_d = nc.dram_tensor("w21b_b", [128, 64], BF16, kind="ExternalInput")
    wsc0_d = nc.dram_tensor("wsc0", [2, 128, 128], BF16, kind="ExternalInput")
    wsc1_d = nc.dram_tensor("wsc1", [2, 128, 64], BF16, kind="ExternalInput")
    outT = nc.dram_tensor("outT", [256, NPC], BF16, kind="ExternalOutput")

    ACT_SILU = mybir.ActivationFunctionType.Silu
    ACT_COPY = mybir.ActivationFunctionType.Copy
    MULT = mybir.AluOpType.mult

    starts = _grid_starts(T)

    with tile.TileContext(nc) as tc:
        with (
            tc.tile_pool(name="const", bufs=1) as cp,
            tc.tile_pool(name="grp", bufs=2) as gp,
            tc.tile_pool(name="hid5", bufs=5) as hp5,
            tc.tile_pool(name="xe", bufs=2) as xep,
            tc.tile_pool(name="st", bufs=3) as sp,
            tc.tile_pool(name="sts", bufs=2) as stsp,
            tc.tile_pool(name="node", bufs=2) as npl,
            tc.tile_pool(name="bank", bufs=1, space="PSUM") as bkp,
            tc.tile_pool(name="wps", bufs=2, space="PSUM") as wpp,
            tc.tile_pool(name="mps", bufs=1, space="PSUM") as mpp,
        ):
            # ---- constants
            wblk1 = cp.tile([128, 128], BF16)
            nc.sync.dma_start(out=wblk1[:], in_=wblk1_d[:])
            wblk2 = cp.tile([128, 128], BF16)
            nc.sync.dma_start(out=wblk2[:], in_=wblk2_d[:])
            wm1 = cp.tile([EDIM, EDIM], BF16)
            nc.sync.dma_start(out=wm1[:], in_=wm1_d[:])
            wbig = cp.tile([EDIM, 256], BF16)
            nc.sync.dma_start(out=wbig[:], in_=wbig_d[:])
            w20a_p = cp.tile([128, 128], BF16)
            nc.sync.dma_start(out=w20a_p[:], in_=w20a_d[:])
            w20b_p = cp.tile([128, 128], BF16)
            nc.sync.dma_start(out=w20b_p[:], in_=w20b_d[:])
            w21a_t = cp.tile([128, 64], BF16)
            nc.sync.dma_start(out=w21a_t[:], in_=w21at_d[:])
            w21b_t = cp.tile([128, 64], BF16)
            nc.sync.dma_start(out=w21b_t[:], in_=w21bt_d[:])
            w21b_b = cp.tile([128, 64], BF16)
            nc.sync.dma_start(out=w21b_b[:], in_=w21bb_d[:])
            wsc0a = cp.tile([128, 128], BF16)
            nc.sync.dma_start(out=wsc0a[:], in_=wsc0_d[0])
            wsc0b = cp.tile([128, 128], BF16)
            nc.sync.dma_start(out=wsc0b[:], in_=wsc0_d[1])
            wsc1a = cp.tile([128, 64], BF16)
            nc.sync.dma_start(out=wsc1a[:], in_=wsc1_d[0])
            wsc1b = cp.tile([128, 64], BF16)
            nc.sync.dma_start(out=wsc1b[:], in_=wsc1_d[1])
            zeros = cp.tile([128, 512], BF16)
            nc.vector.memset(zeros[:], 0.0)

            # ============ radial MLP for all groups (independent of x) ====
            hids = []
            for g in range(NG):
                embt = gp.tile([EDIM, C], BF16, tag="embt")
                nc.sync.dma_start(out=embt[:], in_=embd[g])
                hid = hp5.tile([EDIM, C], BF16, tag="hid", name=f"hid{g}")
                for c0 in range(0, C, 512):
                    sz = min(512, C - c0)
                    hp = wpp.tile([EDIM, 512], F32, tag="wpb")
                    nc.tensor.matmul(hp[:, :sz], lhsT=wm1[:],
                                     rhs=embt[:, c0:c0 + sz],
                                     start=True, stop=True)
                    nc.scalar.activation(hid[:, c0:c0 + sz], hp[:, :sz],
                                         ACT_SILU)
                hids.append(hid)

            # scatter psum banks (reused across groups)
            def bank_tiles():
                return [bkp.tile([128, 512], F32, tag=f"bank{k}",
                                 name=f"bank{k}") for k in range(5)]

            # ================= per-group edge + node phases ================
            cpcnt = [0]
            for g in range(NG):
                banks = bank_tiles()
                if "memset" not in skip:
                    for k in range(5):
                        if k % 2 == 0:
                            nc.vector.memset(banks[k][:], 0.0)
                        else:
                            nc.scalar.activation(banks[k][:], zeros[:],
                                                 ACT_COPY)

                hid = hids[g]

                # per-slot x inputs (channel-major, slot order)
                xe0 = xep.tile([128, C], BF16, tag="xe0", name=f"xe0_{g}")
                nc.sync.dma_start(out=xe0[:], in_=xeT[g][0:128, :])
                xe1 = xep.tile([128, C], BF16, tag="xe1", name=f"xe1_{g}")
                nc.sync.dma_start(out=xe1[:], in_=xeT[g][128:256, :])

                # prefetch node-phase inputs for this group
                cols = slice(g * GRP, (g + 1) * GRP)
                xga = npl.tile([128, 512], BF16, tag="xga")
                nc.sync.dma_start(out=xga[:], in_=xTb[0:128, cols])
                xgb = npl.tile([128, 512], BF16, tag="xgb")
                nc.sync.dma_start(out=xgb[:], in_=xTb[128:256, cols])
                ara = npl.tile([128, 512], BF16, tag="ara")
                nc.sync.dma_start(out=ara[:], in_=arepb[0:128, cols])
                arb = npl.tile([128, 512], BF16, tag="arb")
                nc.sync.dma_start(out=arb[:], in_=arepb[128:256, cols])
                x4s = []
                for d in range(4):
                    x4t = npl.tile([128, 512], BF16, tag=f"x4_{d}",
                                   name=f"x4_{d}_{g}")
                    xsl = xTb[64 * d:64 * d + 64, cols]
                    nc.sync.dma_start(out=x4t[:], in_=AP(
                        xsl.tensor, xsl.offset, [[0, 2]] + xsl.ap))
                    x4s.append(x4t)

                # edge supertiles
                pend = None          # deferred scatter work (prev supertile)

                def emit_scatter(work):
                    st0, L0, wgt0, oht0 = work
                    for lt in ([] if "scatter" in skip else range(L0)):
                        t = st0 + lt
                        col0 = starts[t]
                        wb = lt * 640
                        ob = lt * 4 * SPAN
                        last = (t == T - 1)
                        # bankP: [s0a|s1b_0] <- [A|B0] x oh_a0
                        nc.tensor.matmul(
                            banks[0][:, col0:col0 + SPAN],
                            lhsT=wgt0[:, wb:wb + 128],
                            rhs=oht0[:, ob:ob + SPAN],
                            start=False, stop=last, skip_group_check=True)
                        # bankQ: [s1b_1|s1b_2] <- [B1|B2] x oh_a0
                        nc.tensor.matmul(
                            banks[1][:, col0:col0 + SPAN],
                            lhsT=wgt0[:, wb + 128:wb + 256],
                            rhs=oht0[:, ob:ob + SPAN],
                            start=False, stop=last, skip_group_check=True)
                        # bankR_d: [s1a_d|s0b_d] <- [C|D_d] x oh_a1_d
                        for d in range(3):
                            nc.tensor.matmul(
                                banks[2 + d][:, col0:col0 + SPAN],
                                lhsT=wgt0[:, wb + 256 + 128 * d:
                                         wb + 384 + 128 * d],
                                rhs=oht0[:, ob + SPAN * (1 + d):
                                        ob + SPAN * (2 + d)],
                                start=False, stop=last,
                                skip_group_check=True)

                for st in range(0, T, STW):
                    L = min(STW, T - st)
                    oht = sp.tile([128, STW * 4 * SPAN], BF16, tag="oht")
                    nc.sync.dma_start(
                        out=oht[:, :L * 4 * SPAN],
                        in_=ohd[g][:, st * 4 * SPAN:(st + L) * 4 * SPAN])

                    # per-tile [h(256) | wp(256)] interleaved, bf16
                    hw_sb = sp.tile([128, STW * 512], BF16, tag="hw_sb")

                    # per-tile: lin1 + wp matmuls into one psum bank, then
                    # psum->sbuf copies split across scalar/gpsimd
                    for lt in range(L):
                        t = st + lt
                        hwp = wpp.tile([128, 512], F32, tag="wpb",
                                       name=f"hwp_{g}_{t}")
                        if "lin1" not in skip:
                            nc.tensor.matmul(hwp[:, 0:128],
                                             lhsT=xe0[:, 128 * t:128 * t + 128],
                                             rhs=wblk1[:],
                                             start=True, stop=True)
                            nc.tensor.matmul(hwp[:, 128:256],
                                             lhsT=xe1[:, 128 * t:128 * t + 128],
                                             rhs=wblk2[:],
                                             start=True, stop=True)
                        if "wp" not in skip:
                            nc.tensor.matmul(hwp[:, 256:512],
                                             lhsT=hid[:, 128 * t:128 * t + 128],
                                             rhs=wbig[:],
                                             start=True, stop=True)
                        # psum->sbuf copies (only scalar/DVE may read PSUM);
                        # one [128,512] op per bank, alternating engines
                        dst = hw_sb[:, 512 * lt:512 * lt + 512]
                        if cpcnt[0] % 2 == 0:
                            nc.scalar.activation(dst, hwp[:], ACT_COPY)
                        else:
                            nc.vector.tensor_copy(out=dst, in_=hwp[:])
                        cpcnt[0] += 1

                    # wg products: [A|B0|B1|B2|C|D0|C|D1|C|D2] per tile
                    # layout: 640 cols/tile: A(64) B_d(192) [C|D_d](3x128)
                    wgt = sp.tile([128, STW * 640], BF16, tag="wgt", bufs=2)
                    if "wg" in skip:
                        pass
                    else:
                      # A = w1*g0
                      nc.vector.tensor_tensor(
                        out=_apv(wgt[:], 0, [[640, L], [1, 64]]),
                        in0=_apv(hw_sb[:], 256, [[512, L], [1, 64]]),
                        in1=_apv(hw_sb[:], 0, [[512, L], [1, 64]]),
                        op=MULT)
                      # B_d = w3*g1_d
                      nc.vector.tensor_tensor(
                        out=_apv(wgt[:], 64, [[640, L], [64, 3], [1, 64]]),
                        in0=_apv(hw_sb[:], 256 + 64,
                                 [[512, L], [0, 3], [1, 64]]),
                        in1=_apv(hw_sb[:], 64, [[512, L], [64, 3], [1, 64]]),
                        op=MULT)
                      # C = w2*g0 (replicated 3x at 256+128d)
                      nc.vector.tensor_tensor(
                        out=_apv(wgt[:], 256, [[640, L], [128, 3], [1, 64]]),
                        in0=_apv(hw_sb[:], 256 + 128,
                                 [[512, L], [0, 3], [1, 64]]),
                        in1=_apv(hw_sb[:], 0, [[512, L], [0, 3], [1, 64]]),
                        op=MULT)
                      # D_d = w4'*g1_d (at 320+128d)
                      nc.vector.tensor_tensor(
                        out=_apv(wgt[:], 320, [[640, L], [128, 3], [1, 64]]),
                        in0=_apv(hw_sb[:], 256 + 192,
                                 [[512, L], [0, 3], [1, 64]]),
                        in1=_apv(hw_sb[:], 64, [[512, L], [64, 3], [1, 64]]),
                        op=MULT)

                    # compact scatter matmuls (deferred one supertile so the
                    # PE can pipeline lin1/wp of st+1 with wg of st)
                    if pend is not None:
                        emit_scatter(pend)
                    pend = (st, L, wgt, oht)
                if pend is not None:
                    emit_scatter(pend)

                # drain banks -> sts (bf16)
                sts = []
                for k in range(5):
                    stile = stsp.tile([128, 512], BF16, tag=f"sts{k}",
                                      name=f"sts{k}_{g}")
                    if k % 2 == 0:
                        nc.scalar.activation(stile[:], banks[k][:], ACT_COPY)
                    else:
                        nc.vector.tensor_copy(out=stile[:], in_=banks[k][:])
                    sts.append(stile)
                stsP, stsQ, stsR = sts[0], sts[1], sts[2:5]

                # ---------------- node phase for this group ----------------
                if "node" in skip:
                    outa = npl.tile([128, 512], BF16, tag="outa")
                    nc.vector.tensor_copy(out=outa[:], in_=xga[:])
                    outb = npl.tile([128, 512], BF16, tag="outb")
                    nc.vector.tensor_copy(out=outb[:], in_=xgb[:])
                else:
                    # up0 = W20^T s0 + sc0   [scalars|gates, 512]
                    up0 = bkp.tile([128, 512], F32, tag="bank0",
                                   name=f"up0_{g}")
                    nc.tensor.matmul(up0[:], lhsT=w20a_p[:],
                                     rhs=stsP[:], start=True, stop=False)
                    for d in range(3):
                        nc.tensor.matmul(up0[:], lhsT=w20b_p[:],
                                         rhs=stsR[d][:],
                                         start=False, stop=False)
                    x4 = x4s[0]
                    ya = npl.tile([128, 512], BF16, tag="ya")
                    nc.gpsimd.tensor_tensor(out=ya[:], in0=x4[:], in1=ara[:],
                                            op=MULT)
                    yb = npl.tile([128, 512], BF16, tag="yb")
                    nc.gpsimd.tensor_tensor(out=yb[:], in0=x4[:], in1=arb[:],
                                            op=MULT)
                    nc.tensor.matmul(up0[:], lhsT=wsc0a[:], rhs=ya[:],
                                     start=False, stop=False)
                    nc.tensor.matmul(up0[:], lhsT=wsc0b[:], rhs=yb[:],
                                     start=False, stop=True)

                    # up1: d0 rows 0:64, d1 rows 64:128 of up1a; d2 in up1b
                    up1a = bkp.tile([128, 512], F32, tag="bank1",
                                     name=f"up1a_{g}")
                    for d in (0, 1):
                        rows = slice(64 * d, 64 * d + 64)
                        if d == 0:
                            s1b_src, s1b_w = stsP[:], w21b_b[:]
                        else:
                            s1b_src, s1b_w = stsQ[:], w21b_t[:]
                        nc.tensor.matmul(up1a[rows, :], lhsT=w21a_t[:],
                                         rhs=stsR[d][:],
                                         start=True, stop=False)
                        nc.tensor.matmul(up1a[rows, :], lhsT=s1b_w, rhs=s1b_src,
                                         start=False, stop=False)
                        x4d = x4s[1 + d]
                        yda = npl.tile([128, 512], BF16, tag="yda")
                        nc.gpsimd.tensor_tensor(out=yda[:], in0=x4d[:],
                                                in1=ara[:], op=MULT)
                        ydb = npl.tile([128, 512], BF16, tag="ydb")
                        nc.gpsimd.tensor_tensor(out=ydb[:], in0=x4d[:],
                                                in1=arb[:], op=MULT)
                        nc.tensor.matmul(up1a[rows, :], lhsT=wsc1a[:], rhs=yda[:],
                                         start=False, stop=False)
                        nc.tensor.matmul(up1a[rows, :], lhsT=wsc1b[:], rhs=ydb[:],
                                         start=False, stop=True)

                    # gate scalars/gates -> bf16
                    t0s = npl.tile([128, 512], BF16, tag="t0s")
                    nc.scalar.activation(t0s[:], up0[:], ACT_SILU)

                    up1b = mpp.tile([64, 512], F32, tag="mpsum")
                    nc.tensor.matmul(up1b[:], lhsT=w21a_t[:],
                                     rhs=stsR[2][:], start=True, stop=False)
                    nc.tensor.matmul(up1b[:], lhsT=w21b_b[:],
                                     rhs=stsQ[:], start=False, stop=False)
                    x4d2 = x4s[3]
                    yda2 = npl.tile([128, 512], BF16, tag="yda")
                    nc.gpsimd.tensor_tensor(out=yda2[:], in0=x4d2[:], in1=ara[:],
                                            op=MULT)
                    ydb2 = npl.tile([128, 512], BF16, tag="ydb")
                    nc.gpsimd.tensor_tensor(out=ydb2[:], in0=x4d2[:], in1=arb[:],
                                            op=MULT)
                    nc.tensor.matmul(up1b[:], lhsT=wsc1a[:], rhs=yda2[:],
                                     start=False, stop=False)
                    nc.tensor.matmul(up1b[:], lhsT=wsc1b[:], rhs=ydb2[:],
                                     start=False, stop=True)

                    # assemble: vectors = gates*t1 (t1 from psum); resnet add
                    outa = npl.tile([128, 512], BF16, tag="outa")
                    nc.vector.tensor_add(out=outa[0:64, :], in0=t0s[0:64, :],
                                         in1=xga[0:64, :])
                    nc.vector.tensor_tensor(out=outa[64:128, :],
                                            in0=t0s[64:128, :],
                                            in1=up1a[0:64, :], op=MULT)
                    nc.vector.tensor_add(out=outa[64:128, :], in0=outa[64:128, :],
                                         in1=xga[64:128, :])
                    outb = npl.tile([128, 512], BF16, tag="outb")
                    nc.vector.tensor_tensor(out=outb[0:64, :],
                                            in0=t0s[64:128, :],
                                            in1=up1a[64:128, :], op=MULT)
                    nc.vector.tensor_add(out=outb[0:64, :], in0=outb[0:64, :],
                                         in1=xgb[0:64, :])
                    nc.vector.tensor_tensor(out=outb[64:128, :],
                                            in0=t0s[64:128, :],
                                            in1=up1b[:], op=MULT)
                    nc.vector.tensor_add(out=outb[64:128, :], in0=outb[64:128, :],
                                         in1=xgb[64:128, :])

                nc.sync.dma_start(out=outT[0:128, cols], in_=outa[:])
                nc.sync.dma_start(out=outT[128:256, cols], in_=outb[:])

    nc.compile()
    return nc


# ---------------------------------------------------------------- host prep
def _pack_group(cols, T):
    """Greedy pack of sorted dst-cols into T tiles on the uniform grid.
    Returns per-tile edge index lists (positions into cols) or None."""
    starts = _grid_starts(T)
    res = []
    j, nE = 0, len(cols)
    for t in range(T):
        lo, hi = starts[t], starts[t] + SPAN
        tl = []
        while j < nE and len(tl) < 128 and cols[j] < hi:
            if cols[j] < lo:
                return None
            tl.append(j)
            j += 1
        res.append(tl)
    if j < nE:
        return None
    return res


def _host_prep(node_feats, node_attrs, edge_attrs, edge_embedding,
               W_lin1_0, W_lin1_1, W_mlp1, W_mlp2,
               W_lin2_0, W_lin2_1, W_sc0, W_sc1, edge_index):
    inv = 1.0 / np.sqrt(MUL)
    inv_e = 1.0 / np.sqrt(EDIM)
    inv2 = 1.0 / np.sqrt(2 * MUL)
    inv_n = 1.0 / np.sqrt(AVG_NEIGH)
    inv_sc = 1.0 / np.sqrt(MUL * NZ)

    # channel permutation: ours = [x0(64) | x1 d-major(192)]
    gidx = np.empty(256, np.int64)
    gidx[:64] = np.arange(64)
    for d in range(3):
        for u in range(64):
            gidx[64 + 64 * d + u] = 64 + 3 * u + d

    # permuted node feats in bf16 (row N = zero pad row for empty slots)
    xg_pad = np.zeros((N + 1, 256), np.float32)
    xg_pad[:N] = node_feats[:, gidx]
    xg_pad_b = xg_pad.astype(NP_BF16)

    xgf = np.zeros((NP_PAD, 256), np.float32)
    xgf[:N] = node_feats[:, gidx]
    xT = np.ascontiguousarray(xgf.T)
    xTf = xT.astype(NP_BF16)

    arep_full = np.zeros((256, NP_PAD), np.float32)
    arep_full[:, :N] = np.repeat(node_attrs.T.astype(np.float32), MUL, axis=0)
    arepb_full = arep_full.astype(NP_BF16)

    # ---- edge sorting and per-(core,group) packing
    src = edge_index[0].astype(np.int64)
    dst = edge_index[1].astype(np.int64)
    order = np.argsort(dst, kind="stable")
    src_s, dst_s = src[order], dst[order]
    ea_s = edge_attrs[order].astype(np.float32)
    emb_s = edge_embedding[order].astype(np.float32)

    bounds = np.searchsorted(dst_s, np.arange(0, NP_PAD + 1, GRP))
    all_cols = []
    T = 2
    for c in range(CORES):
        for g in range(NG):
            gi = c * NG + g
            s, e = bounds[gi], bounds[gi + 1]
            cols = (dst_s[s:e] - gi * GRP).astype(int)
            all_cols.append(cols)
            Tg = max(1, int(np.ceil(len(cols) / 128)))
            while Tg < 96 and _pack_group(cols, Tg) is None:
                Tg += 1
            T = max(T, Tg)
    T = T + (T % 2)  # even
    while any(_pack_group(cols, T) is None for cols in all_cols):
        T += 2

    C = T * 128

    per_core = []
    for c in range(CORES):
        slot_src = np.full((NG, C), N, np.int64)   # default -> zero row
        oh = np.zeros((NG, T, 128, 4 * SPAN), np.float32)
        embw = np.zeros((NG, EDIM, C), np.float32)
        starts = _grid_starts(T)
        for g in range(NG):
            gi = c * NG + g
            s = bounds[gi]
            cols = all_cols[gi]
            pk = _pack_group(cols, T)
            assert pk is not None
            for t, tl in enumerate(pk):
                if not tl:
                    continue
                idx = np.asarray(tl, np.int64)
                p = np.arange(len(tl))
                slot = t * 128 + p
                sn = src_s[s + idx]
                slot_src[g, slot] = sn
                embw[g, :, slot] = emb_s[s + idx]
                cc = cols[idx] - starts[t]
                oh[g, t, p, cc] = ea_s[s + idx, 0]               # oh*a0
                for d in range(3):
                    oh[g, t, p, SPAN * (1 + d) + cc] = ea_s[s + idx, 1 + d]
        # pre-gathered x per slot, channel-major [NG, 256, C]
        xe = xg_pad_b[slot_src.reshape(-1)]          # [NG*C, 256] bf16
        xe_dev = np.ascontiguousarray(
            xe.reshape(NG, C, 256).transpose(0, 2, 1))
        # device layout [NG, 128, T*4*SPAN]
        oh_dev = oh.transpose(0, 2, 1, 3).reshape(NG, 128, T * 4 * SPAN)
        per_core.append(dict(
            xeT=xe_dev,
            ohd=np.ascontiguousarray(oh_dev).astype(NP_BF16),
            embd=embw.astype(NP_BF16),
        ))

    # ---- weights
    W10s = (W_lin1_0 * inv).astype(np.float32)
    W11s = (W_lin1_1 * inv).astype(np.float32)
    wblk1 = np.zeros((128, 128), np.float32)
    wblk1[:64, :64] = W10s
    wblk1[64:, 64:] = W11s
    wblk2 = np.zeros((128, 128), np.float32)
    wblk2[:64, :64] = W11s
    wblk2[64:, 64:] = W11s
    wm1 = (W_mlp1 * inv_e).astype(NP_BF16)
    w1 = W_mlp2[:, 0:64]
    w2 = W_mlp2[:, 64:128]
    w3 = W_mlp2[:, 128:192]
    w4 = W_mlp2[:, 192:256]
    # wp cols: [w1 | w3 | w2 | w4']
    wbig = (np.concatenate([w1, w3, w2, w4 * INV_SQRT3], axis=1)
            * inv_e).astype(NP_BF16)
    w20s = (W_lin2_0 * inv2 * inv_n).astype(np.float32)
    w21s = (W_lin2_1 * inv2 * inv_n).astype(np.float32)
    z64x128 = np.zeros((64, 128), np.float32)
    z64x64 = np.zeros((64, 64), np.float32)
    w20a_p = np.concatenate([w20s[0:64], z64x128]).astype(NP_BF16)
    w20b_p = np.concatenate([z64x128, w20s[64:128]]).astype(NP_BF16)
    w21a_t = np.concatenate([w21s[0:64], z64x64]).astype(NP_BF16)
    w21b_t = np.concatenate([w21s[64:128], z64x64]).astype(NP_BF16)
    w21b_b = np.concatenate([z64x64, w21s[64:128]]).astype(NP_BF16)
    wsc0z = (np.transpose(W_sc0, (1, 0, 2)).reshape(NZ * MUL, 2 * MUL)
             * inv_sc).astype(NP_BF16)
    wsc1z = (np.transpose(W_sc1, (1, 0, 2)).reshape(NZ * MUL, MUL)
             * inv_sc).astype(NP_BF16)
    wsc0 = np.stack([wsc0z[:128], wsc0z[128:]])
    wsc1 = np.stack([wsc1z[:128], wsc1z[128:]])

    shared = dict(wblk1=wblk1.astype(NP_BF16),
                  wblk2=wblk2.astype(NP_BF16), wm1=wm1, wbig=wbig,
                  w20a_p=w20a_p, w20b_p=w20b_p, w21a_t=w21a_t,
                  w21b_t=w21b_t, w21b_b=w21b_b, wsc0=wsc0, wsc1=wsc1)
    in_maps = []
    for c in range(CORES):
        m = dict(shared)
        m["xTb"] = np.ascontiguousarray(xTf[:, c * NPC:(c + 1) * NPC])
        m["arepb"] = np.ascontiguousarray(
            arepb_full[:, c * NPC:(c + 1) * NPC])
        m.update(per_core[c])
        in_maps.append(m)
    return T, in_maps, gidx


_PROGRAM_CACHE = {}


def kernel(**inputs):
    global LAST_RESULT
    _install_profile_hook()

    args = {k: np.asarray(v) for k, v in inputs.items()}
    T, in_maps, gidx = _host_prep(
        args["node_feats"].astype(np.float32),
        args["node_attrs"].astype(np.float32),
        args["edge_attrs"].astype(np.float32),
        args["edge_embedding"].astype(np.float32),
        args["W_lin1_0"].astype(np.float32),
        args["W_lin1_1"].astype(np.float32),
        args["W_mlp1"].astype(np.float32),
        args["W_mlp2"].astype(np.float32),
        args["W_lin2_0"].astype(np.float32),
        args["W_lin2_1"].astype(np.float32),
        args["W_sc0"].astype(np.float32),
        args["W_sc1"].astype(np.float32),
        args["edge_index"])

    if T not in _PROGRAM_CACHE:
        _PROGRAM_CACHE[T] = _build_program(T)
    nc = _PROGRAM_CACHE[T]

    trace = bool(int(os.environ.get("BASS_TRACE", "0")))
    res = run_bass_kernel_spmd(nc, in_maps, core_ids=list(range(CORES)),
                               trace=trace)
    LAST_RESULT = res

    outT = np.concatenate(
        [res.results[c]["outT"].astype(np.float32) for c in range(CORES)],
        axis=1)                            # [256, NP_PAD]
    full = outT.T[:N]                      # [N, 256] in our channel order
    out = np.empty((N, 256), np.float32)
    out[:, gidx] = full
    return out


# revision 11
# speedup vs baseline: 1.1547x; 1.1221x over previous
"""Trainium2 Bass kernel for PointConv-style e3nn message passing.

Self-contained: builds + runs an 8-core SPMD Bass kernel via
bass_utils.run_bass_kernel_spmd, accepting FULL inputs and returning the
FULL output.

Design (v3):
- Nodes padded to 20480, split 8 ways (2560/core); edges sorted by dst and
  assigned to the core owning the destination.
- Per core, destinations are processed in 5 groups of 512 nodes. Edges of a
  group are packed into T 128-edge tiles on a uniform column grid (each tile
  owns a baked 32-column window of the group's 512 psum columns), so the
  scatter-add becomes per-tile compact one-hot matmuls into 5 psum banks.
- The a0/a1 spherical-harmonic factors are folded into host-prescaled
  one-hots (oh*a0, oh*a1_d), so the device only forms the w*g products.
- v3 change vs v2: no replicated h-table and no device dma_gather. The host
  pre-gathers x[src] per edge slot (channel-major, slot order) and the
  device computes h = lin1(x) per 128-edge tile with two blockdiag matmuls
  directly into psum, alongside the radial tp-weight matmul.
"""

import os
import sys
import types
import ctypes

import numpy as np

import concourse.bass as bass
import concourse.bacc as bacc
import concourse.tile as tile
from concourse import mybir
from concourse.bass import AP
from concourse.bass_utils import run_bass_kernel_spmd

# ---------------------------------------------------------------- constants
N = 20000
E = 160000
MUL = 64
EDIM = 8
NZ = 4
AVG_NEIGH = 8.0
INV_SQRT3 = float(1.0 / np.sqrt(3.0))

CORES = 8
NP_PAD = 20480            # padded node count
NPC = NP_PAD // CORES     # 2560 nodes per core
GRP = 512                 # nodes per scatter group (psum bank width)
NG = NPC // GRP           # 5 groups per core
SPAN = 32                 # onehot column window per edge tile
BACK = 8                  # grid look-back
STW = 8                   # supertile width (tiles per DVE batch)

F32 = mybir.dt.float32
BF16 = mybir.dt.bfloat16
I32 = mybir.dt.int32
NP_BF16 = mybir.dt.np(mybir.dt.bfloat16)

LAST_RESULT = None


# ------------------------------------------------------- axon profile hook
def _install_profile_hook():
    """Make trace=True / BASS_TRACE=1 work under axon (degrades silently)."""
    if "antenv.axon_hooks" in sys.modules:
        return
    try:
        try:
            from trn_agent_boot.trn_boot import _ntff_profile_via_ctypes
        except ImportError:
            sys.path.insert(0, "/root/.axon_site")
            from trn_agent_boot.trn_boot import _ntff_profile_via_ctypes
        so_path = "/opt/axon/libaxon_pjrt.so"
        lib = ctypes.CDLL(so_path)
        if not hasattr(lib, "axon_start_nrt_profile"):
            return
        hook = _ntff_profile_via_ctypes(so_path)
        mod = types.ModuleType("antenv.axon_hooks")
        state = {"hook": hook}
        mod.set_axon_ntff_profile_hook = lambda h: state.__setitem__("hook", h)
        mod.get_axon_ntff_profile_hook = lambda: state["hook"]
        sys.modules["antenv.axon_hooks"] = mod
        import antenv
        antenv.axon_hooks = mod
    except Exception:
        pass


# ----------------------------------------------- tile-exit drain workaround
def _patch_tile_drain():
    """This toolchain's walrus rejects >1 sem wait on a Drain; hang the exit
    waits on a NoOp chain instead (bacc's generate_event_semaphores then
    legalises them)."""
    from concourse.vector_clock import ScopedClock

    def _drain_and_barrier(self, tick_clock, wait_clock):
        nop_inst = self.nc.sync.nop(nofuse=True, hint="tile_exit_wait")
        wait_clock.add_sem_waits(
            nop_inst.ins, ScopedClock({None: tick_clock.global_clock})
        )
        self.nc.sync.drain()
        self.nc.all_engine_barrier()
        assert self.sems is not None
        popped = self.nc._tile_sem_poison_stack.pop()
        assert popped is self._sem_poison
        self.nc.clear_and_free_semaphores(list(self.sems.allocated().values()))
        self.nc.all_engine_barrier()

    tile.TileContext._drain_and_barrier = _drain_and_barrier


_patch_tile_drain()


def _grid_starts(T):
    return [max(0, min(int(round(i * GRP / T)) - BACK, GRP - SPAN))
            for i in range(T)]


def _apv(base_ap, col_off, dims):
    """AP view of a 2D sbuf/psum tile: partitions x custom free dims."""
    pstep, pcount = base_ap.ap[0]
    return AP(base_ap.tensor, base_ap.offset + col_off,
              [[pstep, pcount]] + dims)


# ---------------------------------------------------------------- program
def _build_program(T):
    """Build the SPMD Bass program for T edge tiles per 512-node group."""
    C = T * 128               # edge slots per group
    skip = set(os.environ.get("BASS_SKIP", "").split(","))

    nc = bacc.Bacc(num_swdge_queues=4)

    # inputs (per core)
    xeT = nc.dram_tensor("xeT", [NG, 256, C], BF16, kind="ExternalInput")
    xTb = nc.dram_tensor("xTb", [256, NPC], BF16, kind="ExternalInput")
    ydram = nc.dram_tensor("ydram", [NG, 128, 8 * 512], BF16,
                           kind="ExternalInput")
    ohd = nc.dram_tensor("ohd", [NG, 128, T * 4 * SPAN], BF16,
                         kind="ExternalInput")
    embd = nc.dram_tensor("embd", [NG, EDIM, C], BF16, kind="ExternalInput")
    wblk1_d = nc.dram_tensor("wblk1", [128, 128], BF16, kind="ExternalInput")
    wblk2_d = nc.dram_tensor("wblk2", [128, 128], BF16, kind="ExternalInput")
    wm1_d = nc.dram_tensor("wm1", [EDIM, EDIM], BF16, kind="ExternalInput")
    wbig_d = nc.dram_tensor("wbig", [EDIM, 256], BF16, kind="ExternalInput")
    w20a_d = nc.dram_tensor("w20a_p", [128, 128], BF16, kind="ExternalInput")
    w20b_d = nc.dram_tensor("w20b_p", [128, 128], BF16, kind="ExternalInput")
    w21at_d = nc.dram_tensor("w21a_t", [128, 64], BF16, kind="ExternalInput")
    w21bt_d = nc.dram_tensor("w21b_t", [128, 64], BF16, kind="ExternalInput")
    w21bb_d = nc.dram_tensor("w21b_b", [128, 64], BF16, kind="ExternalInput")
    wsc0_d = nc.dram_tensor("wsc0", [2, 128, 128], BF16, kind="ExternalInput")
    wsc1_d = nc.dram_tensor("wsc1", [2, 128, 64], BF16, kind="ExternalInput")
    outT = nc.dram_tensor("outT", [256, NPC], BF16, kind="ExternalOutput")

    ACT_SILU = mybir.ActivationFunctionType.Silu
    ACT_COPY = mybir.ActivationFunctionType.Copy
    MULT = mybir.AluOpType.mult

    starts = _grid_starts(T)

    with tile.TileContext(nc) as tc:
        with (
            tc.tile_pool(name="const", bufs=1) as cp,
            tc.tile_pool(name="grp", bufs=2) as gp,
            tc.tile_pool(name="hid5", bufs=5) as hp5,
            tc.tile_pool(name="xe", bufs=2) as xep,
            tc.tile_pool(name="st", bufs=3) as sp,
            tc.tile_pool(name="sts", bufs=2) as stsp,
            tc.tile_pool(name="node", bufs=2) as npl,
            tc.tile_pool(name="bank", bufs=1, space="PSUM") as bkp,
            tc.tile_pool(name="wps", bufs=3, space="PSUM") as wpp,
        ):
            # ---- constants
            wblk1 = cp.tile([128, 128], BF16)
            nc.sync.dma_start(out=wblk1[:], in_=wblk1_d[:])
            wblk2 = cp.tile([128, 128], BF16)
            nc.sync.dma_start(out=wblk2[:], in_=wblk2_d[:])
            wm1 = cp.tile([EDIM, EDIM], BF16)
            nc.sync.dma_start(out=wm1[:], in_=wm1_d[:])
            wbig = cp.tile([EDIM, 256], BF16)
            nc.sync.dma_start(out=wbig[:], in_=wbig_d[:])
            w20a_p = cp.tile([128, 128], BF16)
            nc.sync.dma_start(out=w20a_p[:], in_=w20a_d[:])
            w20b_p = cp.tile([128, 128], BF16)
            nc.sync.dma_start(out=w20b_p[:], in_=w20b_d[:])
            w21a_t = cp.tile([128, 64], BF16)
            nc.sync.dma_start(out=w21a_t[:], in_=w21at_d[:])
            w21b_t = cp.tile([128, 64], BF16)
            nc.sync.dma_start(out=w21b_t[:], in_=w21bt_d[:])
            w21b_b = cp.tile([128, 64], BF16)
            nc.sync.dma_start(out=w21b_b[:], in_=w21bb_d[:])
            wsc0a = cp.tile([128, 128], BF16)
            nc.sync.dma_start(out=wsc0a[:], in_=wsc0_d[0])
            wsc0b = cp.tile([128, 128], BF16)
            nc.sync.dma_start(out=wsc0b[:], in_=wsc0_d[1])
            wsc1a = cp.tile([128, 64], BF16)
            nc.sync.dma_start(out=wsc1a[:], in_=wsc1_d[0])
            wsc1b = cp.tile([128, 64], BF16)
            nc.sync.dma_start(out=wsc1b[:], in_=wsc1_d[1])
            zeros = cp.tile([128, 512], BF16)
            nc.vector.memset(zeros[:], 0.0)

            # ============ radial MLP for all groups (independent of x) ====
            hids = []
            for g in range(NG):
                embt = gp.tile([EDIM, C], BF16, tag="embt")
                nc.sync.dma_start(out=embt[:], in_=embd[g])
                hid = hp5.tile([EDIM, C], BF16, tag="hid", name=f"hid{g}")
                for c0 in range(0, C, 512):
                    sz = min(512, C - c0)
                    hp = wpp.tile([EDIM, 512], F32, tag="wpb")
                    nc.tensor.matmul(hp[:, :sz], lhsT=wm1[:],
                                     rhs=embt[:, c0:c0 + sz],
                                     start=True, stop=True)
                    nc.scalar.activation(hid[:, c0:c0 + sz], hp[:, :sz],
                                         ACT_SILU)
                hids.append(hid)

            # scatter psum banks (reused across groups)
            def bank_tiles():
                return [bkp.tile([128, 512], F32, tag=f"bank{k}",
                                 name=f"bank{k}") for k in range(5)]

            # ================= per-group edge + node phases ================
            cpcnt = [0]
            for g in range(NG):
                banks = bank_tiles()
                if "memset" not in skip:
                    for k in range(5):
                        if k % 2 == 0:
                            nc.vector.memset(banks[k][:], 0.0)
                        else:
                            nc.scalar.activation(banks[k][:], zeros[:],
                                                 ACT_COPY)

                hid = hids[g]

                # per-slot x inputs (channel-major, slot order)
                xe0 = xep.tile([128, C], BF16, tag="xe0", name=f"xe0_{g}")
                nc.sync.dma_start(out=xe0[:], in_=xeT[g][0:128, :])
                xe1 = xep.tile([128, C], BF16, tag="xe1", name=f"xe1_{g}")
                nc.sync.dma_start(out=xe1[:], in_=xeT[g][128:256, :])

                # prefetch node-phase inputs for this group
                cols = slice(g * GRP, (g + 1) * GRP)
                xga = npl.tile([128, 512], BF16, tag="xga")
                nc.sync.dma_start(out=xga[:], in_=xTb[0:128, cols])
                xgb = npl.tile([128, 512], BF16, tag="xgb")
                nc.sync.dma_start(out=xgb[:], in_=xTb[128:256, cols])
                ytile = npl.tile([128, 8 * 512], BF16, tag="ytile")
                nc.sync.dma_start(out=ytile[:], in_=ydram[g])

                # edge supertiles
                pend = None          # deferred scatter work (prev supertile)

                def emit_scatter(work):
                    st0, L0, wgt0, oht0 = work
                    for lt in ([] if "scatter" in skip else range(L0)):
                        t = st0 + lt
                        col0 = starts[t]
                        wb = lt * 640
                        ob = lt * 4 * SPAN
                        last = (t == T - 1)
                        # bankP: [s0a|s1b_0] <- [A|B0] x oh_a0
                        nc.tensor.matmul(
                            banks[0][:, col0:col0 + SPAN],
                            lhsT=wgt0[:, wb:wb + 128],
                            rhs=oht0[:, ob:ob + SPAN],
                            start=False, stop=last, skip_group_check=True)
                        # bankQ: [s1b_1|s1b_2] <- [B1|B2] x oh_a0
                        nc.tensor.matmul(
                            banks[1][:, col0:col0 + SPAN],
                            lhsT=wgt0[:, wb + 128:wb + 256],
                            rhs=oht0[:, ob:ob + SPAN],
                            start=False, stop=last, skip_group_check=True)
                        # bankR_d: [s1a_d|s0b_d] <- [C|D_d] x oh_a1_d
                        for d in range(3):
                            nc.tensor.matmul(
                                banks[2 + d][:, col0:col0 + SPAN],
                                lhsT=wgt0[:, wb + 256 + 128 * d:
                                         wb + 384 + 128 * d],
                                rhs=oht0[:, ob + SPAN * (1 + d):
                                        ob + SPAN * (2 + d)],
                                start=False, stop=last,
                                skip_group_check=True)

                for st in range(0, T, STW):
                    L = min(STW, T - st)
                    oht = sp.tile([128, STW * 4 * SPAN], BF16, tag="oht")
                    nc.sync.dma_start(
                        out=oht[:, :L * 4 * SPAN],
                        in_=ohd[g][:, st * 4 * SPAN:(st + L) * 4 * SPAN])

                    # per-tile [h(256) | wp(256)] interleaved, bf16
                    hw_sb = sp.tile([128, STW * 512], BF16, tag="hw_sb")

                    # per-tile: lin1 + wp matmuls into one psum bank, then
                    # psum->sbuf copies split across scalar/gpsimd
                    for lt in range(L):
                        t = st + lt
                        hwp = wpp.tile([128, 512], F32, tag="wpb",
                                       name=f"hwp_{g}_{t}")
                        if "lin1" not in skip:
                            nc.tensor.matmul(hwp[:, 0:128],
                                             lhsT=xe0[:, 128 * t:128 * t + 128],
                                             rhs=wblk1[:],
                                             start=True, stop=True)
                            nc.tensor.matmul(hwp[:, 128:256],
                                             lhsT=xe1[:, 128 * t:128 * t + 128],
                                             rhs=wblk2[:],
                                             start=True, stop=True)
                        if "wp" not in skip:
                            nc.tensor.matmul(hwp[:, 256:512],
                                             lhsT=hid[:, 128 * t:128 * t + 128],
                                             rhs=wbig[:],
                                             start=True, stop=True)
                        # psum->sbuf copies (only scalar/DVE may read PSUM);
                        # one [128,512] op per bank, alternating engines
                        dst = hw_sb[:, 512 * lt:512 * lt + 512]
                        if cpcnt[0] % 5 in (0, 2, 4):
                            nc.scalar.activation(dst, hwp[:], ACT_COPY)
                        else:
                            nc.vector.tensor_copy(out=dst, in_=hwp[:])
                        cpcnt[0] += 1

                    # wg products: [A|B0|B1|B2|C|D0|C|D1|C|D2] per tile
                    # layout: 640 cols/tile: A(64) B_d(192) [C|D_d](3x128)
                    wgt = sp.tile([128, STW * 640], BF16, tag="wgt", bufs=2)
                    if "wg" in skip:
                        pass
                    else:
                      # A = w1*g0 (gpsimd, sbuf only)
                      nc.gpsimd.tensor_tensor(
                        out=_apv(wgt[:], 0, [[640, L], [1, 64]]),
                        in0=_apv(hw_sb[:], 256, [[512, L], [1, 64]]),
                        in1=_apv(hw_sb[:], 0, [[512, L], [1, 64]]),
                        op=MULT)
                      # B_d = w3*g1_d
                      nc.vector.tensor_tensor(
                        out=_apv(wgt[:], 64, [[640, L], [64, 3], [1, 64]]),
                        in0=_apv(hw_sb[:], 256 + 64,
                                 [[512, L], [0, 3], [1, 64]]),
                        in1=_apv(hw_sb[:], 64, [[512, L], [64, 3], [1, 64]]),
                        op=MULT)
                      # C = w2*g0 (replicated 3x at 256+128d)
                      nc.vector.tensor_tensor(
                        out=_apv(wgt[:], 256, [[640, L], [128, 3], [1, 64]]),
                        in0=_apv(hw_sb[:], 256 + 128,
                                 [[512, L], [0, 3], [1, 64]]),
                        in1=_apv(hw_sb[:], 0, [[512, L], [0, 3], [1, 64]]),
                        op=MULT)
                      # D_d = w4'*g1_d (at 320+128d)
                      nc.vector.tensor_tensor(
                        out=_apv(wgt[:], 320, [[640, L], [128, 3], [1, 64]]),
                        in0=_apv(hw_sb[:], 256 + 192,
                                 [[512, L], [0, 3], [1, 64]]),
                        in1=_apv(hw_sb[:], 64, [[512, L], [64, 3], [1, 64]]),
                        op=MULT)

                    # compact scatter matmuls (deferred one supertile so the
                    # PE can pipeline lin1/wp of st+1 with wg of st)
                    if pend is not None:
                        emit_scatter(pend)
                    pend = (st, L, wgt, oht)
                if pend is not None:
                    emit_scatter(pend)

                # drain banks -> sts (bf16)
                sts = []
                for k in range(5):
                    stile = stsp.tile([128, 512], BF16, tag=f"sts{k}",
                                      name=f"sts{k}_{g}")
                    if k % 2 == 0:
                        nc.scalar.activation(stile[:], banks[k][:], ACT_COPY)
                    else:
                        nc.vector.tensor_copy(out=stile[:], in_=banks[k][:])
                    sts.append(stile)
                stsP, stsQ, stsR = sts[0], sts[1], sts[2:5]

                # ---------------- node phase for this group ----------------
                if "node" in skip:
                    outa = npl.tile([128, 512], BF16, tag="outa")
                    nc.vector.tensor_copy(out=outa[:], in_=xga[:])
                    outb = npl.tile([128, 512], BF16, tag="outb")
                    nc.vector.tensor_copy(out=outb[:], in_=xgb[:])
                else:
                    # up0 = W20^T s0 + sc0   [scalars|gates, 512]
                    up0 = bkp.tile([128, 512], F32, tag="bank0",
                                   name=f"up0_{g}")
                    nc.tensor.matmul(up0[:], lhsT=w20a_p[:],
                                     rhs=stsP[:], start=True, stop=False)
                    for d in range(3):
                        nc.tensor.matmul(up0[:], lhsT=w20b_p[:],
                                         rhs=stsR[d][:],
                                         start=False, stop=False)
                    nc.tensor.matmul(up0[:], lhsT=wsc0a[:],
                                     rhs=ytile[:, 0:512],
                                     start=False, stop=False)
                    nc.tensor.matmul(up0[:], lhsT=wsc0b[:],
                                     rhs=ytile[:, 512:1024],
                                     start=False, stop=True)

                    # up1: d0 rows 0:64, d1 rows 64:128 of up1a; d2 in up1b
                    up1a = bkp.tile([128, 512], F32, tag="bank1",
                                     name=f"up1a_{g}")
                    for d in (0, 1):
                        rows = slice(64 * d, 64 * d + 64)
                        if d == 0:
                            s1b_src, s1b_w = stsP[:], w21b_b[:]
                        else:
                            s1b_src, s1b_w = stsQ[:], w21b_t[:]
                        nc.tensor.matmul(up1a[rows, :], lhsT=w21a_t[:],
                                         rhs=stsR[d][:],
                                         start=True, stop=False)
                        nc.tensor.matmul(up1a[rows, :], lhsT=s1b_w, rhs=s1b_src,
                                         start=False, stop=False)
                        yo = 1024 * (1 + d)
                        nc.tensor.matmul(up1a[rows, :], lhsT=wsc1a[:],
                                         rhs=ytile[:, yo:yo + 512],
                                         start=False, stop=False)
                        nc.tensor.matmul(up1a[rows, :], lhsT=wsc1b[:],
                                         rhs=ytile[:, yo + 512:yo + 1024],
                                         start=False, stop=True)

                    # gate scalars/gates -> bf16
                    t0s = npl.tile([128, 512], BF16, tag="t0s")
                    nc.scalar.activation(t0s[:], up0[:], ACT_SILU)

                    up1b = wpp.tile([64, 512], F32, tag="wpb",
                                    name=f"up1b_{g}")
                    nc.tensor.matmul(up1b[:], lhsT=w21a_t[:],
                                     rhs=stsR[2][:], start=True, stop=False)
                    nc.tensor.matmul(up1b[:], lhsT=w21b_b[:],
                                     rhs=stsQ[:], start=False, stop=False)
                    nc.tensor.matmul(up1b[:], lhsT=wsc1a[:],
                                     rhs=ytile[:, 3072:3584],
                                     start=False, stop=False)
                    nc.tensor.matmul(up1b[:], lhsT=wsc1b[:],
                                     rhs=ytile[:, 3584:4096],
                                     start=False, stop=True)

                    # assemble: vectors = gates*t1 (t1 from psum); resnet add
                    outa = npl.tile([128, 512], BF16, tag="outa")
                    nc.vector.tensor_add(out=outa[0:64, :], in0=t0s[0:64, :],
                                         in1=xga[0:64, :])
                    nc.vector.tensor_tensor(out=outa[64:128, :],
                                            in0=t0s[64:128, :],
                                            in1=up1a[0:64, :], op=MULT)
                    nc.vector.tensor_add(out=outa[64:128, :], in0=outa[64:128, :],
                                         in1=xga[64:128, :])
                    outb = npl.tile([128, 512], BF16, tag="outb")
                    nc.vector.tensor_tensor(out=outb[0:64, :],
                                            in0=t0s[64:128, :],
                                            in1=up1a[64:128, :], op=MULT)
                    nc.vector.tensor_add(out=outb[0:64, :], in0=outb[0:64, :],
                                         in1=xgb[0:64, :])
                    nc.vector.tensor_tensor(out=outb[64:128, :],
                                            in0=t0s[64:128, :],
                                            in1=up1b[:], op=MULT)
                    nc.vector.tensor_add(out=outb[64:128, :], in0=outb[64:128, :],
                                         in1=xgb[64:128, :])

                nc.sync.dma_start(out=outT[0:128, cols], in_=outa[:])
                nc.sync.dma_start(out=outT[128:256, cols], in_=outb[:])

    nc.compile()
    return nc


# ---------------------------------------------------------------- host prep
def _pack_group(cols, T):
    """Greedy pack of sorted dst-cols into T tiles on the uniform grid.
    Returns per-tile edge index lists (positions into cols) or None."""
    starts = _grid_starts(T)
    res = []
    j, nE = 0, len(cols)
    for t in range(T):
        lo, hi = starts[t], starts[t] + SPAN
        tl = []
        while j < nE and len(tl) < 128 and cols[j] < hi:
            if cols[j] < lo:
                return None
            tl.append(j)
            j += 1
        res.append(tl)
    if j < nE:
        return None
    return res


def _host_prep(node_feats, node_attrs, edge_attrs, edge_embedding,
               W_lin1_0, W_lin1_1, W_mlp1, W_mlp2,
               W_lin2_0, W_lin2_1, W_sc0, W_sc1, edge_index):
    inv = 1.0 / np.sqrt(MUL)
    inv_e = 1.0 / np.sqrt(EDIM)
    inv2 = 1.0 / np.sqrt(2 * MUL)
    inv_n = 1.0 / np.sqrt(AVG_NEIGH)
    inv_sc = 1.0 / np.sqrt(MUL * NZ)

    # channel permutation: ours = [x0(64) | x1 d-major(192)]
    gidx = np.empty(256, np.int64)
    gidx[:64] = np.arange(64)
    for d in range(3):
        for u in range(64):
            gidx[64 + 64 * d + u] = 64 + 3 * u + d

    # permuted node feats in bf16 (row N = zero pad row for empty slots)
    xg_pad = np.zeros((N + 1, 256), np.float32)
    xg_pad[:N] = node_feats[:, gidx]
    xg_pad_b = xg_pad.astype(NP_BF16)

    xgf = np.zeros((NP_PAD, 256), np.float32)
    xgf[:N] = node_feats[:, gidx]
    xT = np.ascontiguousarray(xgf.T)
    xTf = xT.astype(NP_BF16)

    attT = np.zeros((NZ, NP_PAD), np.float32)
    attT[:, :N] = node_attrs.T.astype(np.float32)

    # ---- edge sorting and per-(core,group) packing
    src = edge_index[0].astype(np.int64)
    dst = edge_index[1].astype(np.int64)
    order = np.argsort(dst, kind="stable")
    src_s, dst_s = src[order], dst[order]
    ea_s = edge_attrs[order].astype(np.float32)
    emb_s = edge_embedding[order].astype(np.float32)

    bounds = np.searchsorted(dst_s, np.arange(0, NP_PAD + 1, GRP))
    all_cols = []
    T = 2
    for c in range(CORES):
        for g in range(NG):
            gi = c * NG + g
            s, e = bounds[gi], bounds[gi + 1]
            cols = (dst_s[s:e] - gi * GRP).astype(int)
            all_cols.append(cols)
            Tg = max(1, int(np.ceil(len(cols) / 128)))
            while Tg < 96 and _pack_group(cols, Tg) is None:
                Tg += 1
            T = max(T, Tg)
    T = T + (T % 2)  # even
    while any(_pack_group(cols, T) is None for cols in all_cols):
        T += 2

    C = T * 128

    per_core = []
    for c in range(CORES):
        slot_src = np.full((NG, C), N, np.int64)   # default -> zero row
        oh = np.zeros((NG, T, 128, 4 * SPAN), np.float32)
        embw = np.zeros((NG, EDIM, C), np.float32)
        starts = _grid_starts(T)
        for g in range(NG):
            gi = c * NG + g
            s = bounds[gi]
            cols = all_cols[gi]
            pk = _pack_group(cols, T)
            assert pk is not None
            for t, tl in enumerate(pk):
                if not tl:
                    continue
                idx = np.asarray(tl, np.int64)
                p = np.arange(len(tl))
                slot = t * 128 + p
                sn = src_s[s + idx]
                slot_src[g, slot] = sn
                embw[g, :, slot] = emb_s[s + idx]
                cc = cols[idx] - starts[t]
                oh[g, t, p, cc] = ea_s[s + idx, 0]               # oh*a0
                for d in range(3):
                    oh[g, t, p, SPAN * (1 + d) + cc] = ea_s[s + idx, 1 + d]
        # pre-gathered x per slot, channel-major [NG, 256, C]
        xe = xg_pad_b[slot_src.reshape(-1)]          # [NG*C, 256] bf16
        xe_dev = np.ascontiguousarray(
            xe.reshape(NG, C, 256).transpose(0, 2, 1))
        # device layout [NG, 128, T*4*SPAN]
        oh_dev = oh.transpose(0, 2, 1, 3).reshape(NG, 128, T * 4 * SPAN)
        per_core.append(dict(
            xeT=xe_dev,
            ohd=np.ascontiguousarray(oh_dev).astype(NP_BF16),
            embd=embw.astype(NP_BF16),
        ))

    # ---- weights
    W10s = (W_lin1_0 * inv).astype(np.float32)
    W11s = (W_lin1_1 * inv).astype(np.float32)
    wblk1 = np.zeros((128, 128), np.float32)
    wblk1[:64, :64] = W10s
    wblk1[64:, 64:] = W11s
    wblk2 = np.zeros((128, 128), np.float32)
    wblk2[:64, :64] = W11s
    wblk2[64:, 64:] = W11s
    wm1 = (W_mlp1 * inv_e).astype(NP_BF16)
    w1 = W_mlp2[:, 0:64]
    w2 = W_mlp2[:, 64:128]
    w3 = W_mlp2[:, 128:192]
    w4 = W_mlp2[:, 192:256]
    # wp cols: [w1 | w3 | w2 | w4']
    wbig = (np.concatenate([w1, w3, w2, w4 * INV_SQRT3], axis=1)
            * inv_e).astype(NP_BF16)
    w20s = (W_lin2_0 * inv2 * inv_n).astype(np.float32)
    w21s = (W_lin2_1 * inv2 * inv_n).astype(np.float32)
    z64x128 = np.zeros((64, 128), np.float32)
    z64x64 = np.zeros((64, 64), np.float32)
    w20a_p = np.concatenate([w20s[0:64], z64x128]).astype(NP_BF16)
    w20b_p = np.concatenate([z64x128, w20s[64:128]]).astype(NP_BF16)
    w21a_t = np.concatenate([w21s[0:64], z64x64]).astype(NP_BF16)
    w21b_t = np.concatenate([w21s[64:128], z64x64]).astype(NP_BF16)
    w21b_b = np.concatenate([z64x64, w21s[64:128]]).astype(NP_BF16)
    wsc0z = (np.transpose(W_sc0, (1, 0, 2)).reshape(NZ * MUL, 2 * MUL)
             * inv_sc).astype(NP_BF16)
    wsc1z = (np.transpose(W_sc1, (1, 0, 2)).reshape(NZ * MUL, MUL)
             * inv_sc).astype(NP_BF16)
    wsc0 = np.stack([wsc0z[:128], wsc0z[128:]])
    wsc1 = np.stack([wsc1z[:128], wsc1z[128:]])

    shared = dict(wblk1=wblk1.astype(NP_BF16),
                  wblk2=wblk2.astype(NP_BF16), wm1=wm1, wbig=wbig,
                  w20a_p=w20a_p, w20b_p=w20b_p, w21a_t=w21a_t,
                  w21b_t=w21b_t, w21b_b=w21b_b, wsc0=wsc0, wsc1=wsc1)
    in_maps = []
    for c in range(CORES):
        m = dict(shared)
        csl = slice(c * NPC, (c + 1) * NPC)
        m["xTb"] = np.ascontiguousarray(xTf[:, csl])
        # host-precomputed self-connection inputs y = x_block * z_attr
        # block (k, h): cols 512*(2k+h), rows z*64+u (z = 2h + p//64)
        ycore = np.empty((128, NPC, 8), np.float32)
        xTc = xT[:, csl]
        atc = attT[:, csl]
        for k in range(4):
            xk = xTc[64 * k:64 * k + 64]
            for h in range(2):
                ycore[0:64, :, 2 * k + h] = xk * atc[2 * h]
                ycore[64:128, :, 2 * k + h] = xk * atc[2 * h + 1]
        yd = ycore.reshape(128, NG, 512, 8).transpose(1, 0, 3, 2).reshape(
            NG, 128, 8 * 512)
        m["ydram"] = np.ascontiguousarray(yd).astype(NP_BF16)
        m.update(per_core[c])
        in_maps.append(m)
    return T, in_maps, gidx


_PROGRAM_CACHE = {}


def kernel(**inputs):
    global LAST_RESULT
    _install_profile_hook()

    args = {k: np.asarray(v) for k, v in inputs.items()}
    T, in_maps, gidx = _host_prep(
        args["node_feats"].astype(np.float32),
        args["node_attrs"].astype(np.float32),
        args["edge_attrs"].astype(np.float32),
        args["edge_embedding"].astype(np.float32),
        args["W_lin1_0"].astype(np.float32),
        args["W_lin1_1"].astype(np.float32),
        args["W_mlp1"].astype(np.float32),
        args["W_mlp2"].astype(np.float32),
        args["W_lin2_0"].astype(np.float32),
        args["W_lin2_1"].astype(np.float32),
        args["W_sc0"].astype(np.float32),
        args["W_sc1"].astype(np.float32),
        args["edge_index"])

    if T not in _PROGRAM_CACHE:
        _PROGRAM_CACHE[T] = _build_program(T)
    nc = _PROGRAM_CACHE[T]

    trace = bool(int(os.environ.get("BASS_TRACE", "0")))
    res = run_bass_kernel_spmd(nc, in_maps, core_ids=list(range(CORES)),
                               trace=trace)
    LAST_RESULT = res

    outT = np.concatenate(
        [res.results[c]["outT"].astype(np.float32) for c in range(CORES)],
        axis=1)                            # [256, NP_PAD]
    full = outT.T[:N]                      # [N, 256] in our channel order
    out = np.empty((N, 256), np.float32)
    out[:, gidx] = full
    return out
